# revision 38
# baseline (speedup 1.0000x reference)
"""Trainium2 Bass kernel for a dense pre-LN transformer block (B=2, T=2048, C=1024, H=16).

Sharding: zero-collective sequence parallelism over 8 cores. Core c handles
batch b=c//4 and query tiles (slots) {r, 7-r, 8+r, 15-r} (r=c%4, 128 rows
each). The slot-position windows align with 4-tile causal bands: slot
position i always has its diagonal inside key tiles [4i, 4i+4), so the
program is SPMD-identical while masks are data (paged-mask offsets).

Per head, attention runs in two parts: part A covers query cols 0:256
(slots 0,1; key tiles 0..7), part B covers cols 256:512 (slots 2,3; key
tiles 0..15), so part A can start once k/v tiles 0..7 are layer-normed.
Heads are processed in PAIRS with QK matmuls interleaved between the two
heads: the pair occupies disjoint 64-row groups of the PE array, so both
the LDWEIGHTS and the MATMULs of the two heads run concurrently (~2x QK).
Causality: exact spans per chunk; only the 4-tile diagonal bands get a
bf16 mask multiply. Softmax denominators ride along as an extra
ones-column in the v stationary; their reciprocals (fast DVE approx) and
the s_bf gather are hoisted into the part-B pipeline so c_proj can start
immediately after attention. LN2 token-stat matmuls accumulate per-ot
inside c_proj. Activation tables are pinned so Exp/Ln share one set
(zero mid-kernel table swaps except one load for gelu).

Numerics: all matmuls bf16 with fp32 PSUM accumulation; residuals/LN math
fp32 (fp8 was evaluated and rejected: e4m3 matmul quantization costs
1.3-2e-2 of max-rel-err against a 2e-2 budget).
"""

import sys
import functools

sys.path.insert(0, "/opt/trn_rl_repo")

import numpy as np
import ml_dtypes

import concourse.bass as bass
import concourse.bacc as bacc
import concourse.mybir as mybir
import concourse.tile as tile
from concourse.bass_utils import run_bass_kernel_spmd

# Pin Exp/Ln to the one table set that holds both, so the act-table
# placement pass cannot thrash between exp_and_others and natural_log
# (each swap costs ~1.3us and stalls the attention softmax pipeline).
# Indices into act_info.json are preserved; only set membership as seen
# by the placement pass is narrowed.
_AF = mybir.ActivationFunctionType


@functools.cache
def _pinned_act_tables(arch):
    import concourse.hw_specs as hw_specs
    out = {}
    for name, fns in hw_specs.get_activation_tables(arch).items():
        fns = set(fns)
        if name != "natural_log_exp_and_others":
            fns.discard(_AF.Exp)
            fns.discard(_AF.Ln)
        out[name] = fns
    return out


bacc.get_activation_tables = _pinned_act_tables

F32 = mybir.dt.float32
BF16 = mybir.dt.bfloat16
AF = mybir.ActivationFunctionType
ALU = mybir.AluOpType

B, T, C, H, D = 2, 2048, 1024, 16, 64
NT = T // 128          # 16 key tiles
NC = C // 128          # 8 channel tiles
NF = 4 * C // 128      # 32 fc tiles
NSLOT = 4
N_CORES = 8
EPS = 1e-5
SCALE = 1.0 / 8.0      # 1/sqrt(D)

_CACHE = {}


def build():
    nc = bacc.Bacc("TRN2", target_bir_lowering=False, debug=False,
                   num_devices=N_CORES)

    q_d = nc.dram_tensor("q_s", [NSLOT, 128, C], BF16, kind="ExternalInput")
    k_d = nc.dram_tensor("k_f", [NT, 128, C], BF16, kind="ExternalInput")
    v_d = nc.dram_tensor("v_f", [NT, 128, C], BF16, kind="ExternalInput")
    mask_d = nc.dram_tensor("mask", [128, 2, 4, 256], BF16, kind="ExternalInput")
    vecs_d = nc.dram_tensor("vecs", [C, 6], F32, kind="ExternalInput")
    w1_d = nc.dram_tensor("w1row", [1, C], BF16, kind="ExternalInput")
    cpw_d = nc.dram_tensor("cpwt", [128, NC, C], BF16, kind="ExternalInput")
    fcw_d = nc.dram_tensor("fcwt", [NF, 128, NC, 128], BF16, kind="ExternalInput")
    pjw_d = nc.dram_tensor("pjwt", [NC, 128, NF, 128], BF16, kind="ExternalInput")
    fcb_d = nc.dram_tensor("fcb", [4 * C], F32, kind="ExternalInput")
    out_d = nc.dram_tensor("out", [NSLOT, 128, C], F32, kind="ExternalOutput")

    with tile.TileContext(nc) as tc:
      with tc.tile_pool(name="pg", bufs=1) as pg:
        # ---- persistent constants (gpsimd DMA queue: keeps the sync
        # queue free for the startup q/k/v tile loads) ----
        vecs = pg.tile([128, NC, 6], F32)   # ln1w ln1b apb pjb w2 b2
        nc.gpsimd.dma_start(vecs[:], vecs_d.ap().rearrange("(ct p) v -> p ct v", p=128))
        masks = pg.tile([128, 2, 4, 256], BF16)
        nc.gpsimd.dma_start(masks[:], mask_d.ap())
        w1_bf = pg.tile([1, C], BF16)
        nc.gpsimd.dma_start(w1_bf[:], w1_d.ap())
        fcb = pg.tile([128, NF], F32)
        nc.gpsimd.dma_start(fcb[:], fcb_d.ap().rearrange("(ft p) -> p ft", p=128))

        ones_f = pg.tile([128, 128], F32)
        nc.gpsimd.memset(ones_f[:], 1.0)
        ident = pg.tile([128, 128], F32)
        nc.gpsimd.affine_select(ident[:], ones_f[:], [[1, 128]], ALU.is_equal,
                                0.0, channel_multiplier=-1)
        ones128_bf = pg.tile([128, 128], BF16)
        nc.gpsimd.memset(ones128_bf[:], 1.0)
        ident_bf = pg.tile([128, 128], BF16)
        nc.gpsimd.affine_select(ident_bf[:], ones128_bf[:], [[1, 128]], ALU.is_equal,
                                0.0, channel_multiplier=-1)
        ones_bf = pg.tile([128, 1], BF16)
        nc.gpsimd.memset(ones_bf[:], 1.0)
        ones_row = pg.tile([1, 128], BF16)
        nc.gpsimd.memset(ones_row[:], 1.0)

        ln1w = lambda ct: vecs[:, ct, 0:1]
        ln1b = lambda ct: vecs[:, ct, 1:2]
        apb = lambda ct: vecs[:, ct, 2:3]
        pjb = lambda ct: vecs[:, ct, 3:4]
        w2c = lambda ct: vecs[:, ct, 4:5]
        b2c = lambda ct: vecs[:, ct, 5:6]

        # ---- cross-phase tensors ----
        qT_bf = pg.tile([128, NC, 512], BF16)   # LN1(q)^T w/ w,b (QK rhs + residual)
        xT = pg.tile([128, NC, 512], F32)       # attn residual output (C-major)

        py_cm = tc.tile_pool(name="py", bufs=1)
        py = py_cm.__enter__()
        yT_all = py.tile([128, NC, 512], F32)   # raw attention out (pre 1/s, w1, b1)
        s_all = py.tile([H, 512], F32)          # softmax denominators
        s_bf = py.tile([1, H * 512], BF16)      # reciprocals, gathered on one row
        srec_b = py.tile([H, 512], BF16)
        s_allB2 = py.tile([8, 256], F32)        # part-B denoms, heads 8..15
        srec_b2 = py.tile([8, 256], BF16)
        sA_rec = py.tile([H, 256], F32)         # fast-recip scratch (part A)
        cpw_sb = py.tile([128, NC, C], BF16)    # c_proj weights, preloaded
        nc.gpsimd.dma_start(cpw_sb[:], cpw_d.ap())

        with tc.tile_pool(name="pa", bufs=1) as pa:
            kT = pa.tile([128, NC, T], BF16)          # LN1(k)^T w/ w,b
            # v_ext cols: [v0..v63 | ones] — AV output rows 0..63 = y,
            # row 64 = softmax denominator.
            v_ext = pa.tile([128, NT, H, 65], BF16)
            for tt in range(NT):
                nc.gpsimd.memset(v_ext[:, tt, :, 64:65], 1.0)

            with (
                tc.tile_pool(name="pln", bufs=8) as pl,
                tc.tile_pool(name="plz", bufs=2) as plz,
                tc.tile_pool(name="pla", bufs=3) as pla,
                tc.tile_pool(name="pat", bufs=2) as pat,
                tc.tile_pool(name="psA", bufs=3, space="PSUM") as psA,
                tc.tile_pool(name="psY", bufs=2, space="PSUM") as psY,
            ):
                # ---------- LN1 group: load, stats, rstd, normalize ----------
                def ln_group(src_d, tts, kind):
                    n = len(tts)
                    xs = []
                    agg = pla.tile([128, 8, 2], F32, tag="agg")
                    for gi, tt in enumerate(tts):
                        x = pl.tile([128, C], BF16, tag="xin")
                        nc.sync.dma_start(x[:], src_d.ap()[tt])
                        st6 = pl.tile([128, 2, 6], F32, tag="st6")
                        nc.vector.bn_stats(st6[:, 0, :], x[:, 0:512])
                        nc.vector.bn_stats(st6[:, 1, :], x[:, 512:1024])
                        nc.vector.bn_aggr(agg[:, gi, :], st6[:])
                        xs.append(x)
                    veps = pla.tile([128, 8], F32, tag="veps")
                    nc.vector.tensor_scalar(veps[:, 0:n], agg[:, 0:n, 1], EPS, None,
                                            ALU.add)
                    rstd = pla.tile([128, 8], F32, tag="rstd")
                    nc.scalar.activation(rstd[:, 0:n], veps[:, 0:n], AF.Ln)
                    nc.scalar.activation(rstd[:, 0:n], rstd[:, 0:n], AF.Exp,
                                         scale=-0.5)
                    nmr = pla.tile([128, 8], F32, tag="nmr")
                    nc.vector.tensor_tensor(nmr[:, 0:n], agg[:, 0:n, 0], rstd[:, 0:n],
                                            ALU.mult)
                    nc.vector.tensor_scalar(nmr[:, 0:n], nmr[:, 0:n], -1.0, None,
                                            ALU.mult)
                    if kind == "v":
                        for gi, tt in enumerate(tts):
                            nc.vector.tensor_scalar(
                                v_ext[:, tt, :, 0:64],
                                xs[gi][:].rearrange("p (h d) -> p h d", h=H),
                                rstd[:, gi:gi + 1], nmr[:, gi:gi + 1],
                                ALU.mult, ALU.add)
                        return
                    # q/k: normalize -> transpose -> evacuate with w,b
                    dstT, col0 = (qT_bf, 0) if kind == "q" else (kT, tts[0] * 128)
                    late = kind == "k" and tts[0] >= 8
                    zs = []
                    for gi, tt in enumerate(tts):
                        z = plz.tile([128, C], BF16, tag=f"z{gi % 4}")
                        nc.vector.tensor_scalar(z[:], xs[gi][:],
                                                rstd[:, gi:gi + 1], nmr[:, gi:gi + 1],
                                                ALU.mult, ALU.add)
                        zs.append(z)
                    for half in range(n // 4):
                        for ct in range(NC):
                            ps = psA.tile([128, 4, 256], F32, name="ps",
                                          tag="sc256")
                            pv = ps[:].bitcast(BF16)[:, :, 0:128]
                            for gi in range(4):
                                nc.tensor.transpose(
                                    pv[:, gi, :],
                                    zs[half * 4 + gi][:, ct * 128:(ct + 1) * 128],
                                    ident_bf[:])
                            dst = dstT[:, ct, col0 + half * 512:col0 + half * 512 + 512]
                            if late and ct % 2 == 0:
                                # split evacs ACT/DVE while softmax runs
                                nc.vector.tensor_scalar(dst, pv[:], ln1w(ct),
                                                        ln1b(ct), ALU.mult, ALU.add)
                            else:
                                nc.scalar.activation(dst, pv[:], AF.Identity,
                                                     bias=ln1b(ct), scale=ln1w(ct))

                # ---------- attention, head pair (h0, h0+1) ----------
                # QK matmuls interleave the two heads (disjoint 64-row PE
                # groups -> concurrent LDWEIGHTS+MATMUL for the pair).
                def qk_a_pair(h0):
                    hs = (h0, h0 + 1)
                    sc0 = {}
                    for h in hs:
                        sc0[h] = psA.tile([128, 4, 256], F32, name="sc0", tag="sc256")
                    for t in range(4):
                        for h in hs:
                            ct, sel = h // 2, (h % 2) * 64
                            nc.tensor.matmul(sc0[h][:, t, :],
                                             kT[sel:sel + 64, ct, t * 128:(t + 1) * 128],
                                             qT_bf[sel:sel + 64, ct, 0:256],
                                             tile_position=(sel, 0),
                                             skip_group_check=True)
                    att0 = {}
                    for h in hs:
                        a = pat.tile([128, 4, 256], BF16, tag="attA0")
                        nc.scalar.activation(a[:], sc0[h][:], AF.Exp, scale=SCALE)
                        nc.vector.tensor_tensor(a[:, :, 0:128], a[:, :, 0:128],
                                                masks[:, 0, :, 0:128], ALU.mult)
                        att0[h] = a
                    sc1 = {}
                    for h in hs:
                        sc1[h] = psA.tile([128, 4, 256], F32, name="sc1", tag="sc256")
                    for t in range(4, 8):
                        for h in hs:
                            ct, sel = h // 2, (h % 2) * 64
                            nc.tensor.matmul(sc1[h][:, t - 4, 0:128],
                                             kT[sel:sel + 64, ct, t * 128:(t + 1) * 128],
                                             qT_bf[sel:sel + 64, ct, 128:256],
                                             tile_position=(sel, 0),
                                             skip_group_check=True)
                    att1 = {}
                    for h in hs:
                        a = pat.tile([128, 4, 128], BF16, tag="attA1")
                        nc.scalar.activation(a[:], sc1[h][:, :, 0:128], AF.Exp,
                                             scale=SCALE)
                        nc.vector.tensor_tensor(a[:], a[:],
                                                masks[:, 0, :, 128:256], ALU.mult)
                        att1[h] = a
                    return {h: (att0[h], att1[h]) for h in hs}

                def av_a(h, att0, att1):
                    yp = psY.tile([65, 512], F32, tag="yp")
                    vx = lambda t: v_ext[:, t, h, :]
                    for t in range(3):
                        nc.tensor.matmul(yp[:, 0:256], vx(t), att0[:, t, :],
                                         start=(t == 0), stop=False,
                                         skip_group_check=True)
                    nc.tensor.matmul(yp[:, 0:128], vx(3), att0[:, 3, 0:128],
                                     start=False, stop=True, skip_group_check=True)
                    nc.tensor.matmul(yp[:, 128:256], vx(3), att0[:, 3, 128:256],
                                     start=False, stop=False, skip_group_check=True)
                    for t in range(4, 8):
                        nc.tensor.matmul(yp[:, 128:256], vx(t), att1[:, t - 4, :],
                                         start=False, stop=(t == 7),
                                         skip_group_check=True)
                    return yp

                def qk_b_pair(h0):
                    hs = (h0, h0 + 1)
                    att0 = {h: pat.tile([128, 8, 256], BF16, name="attB0", tag="attB0") for h in hs}
                    for half in range(2):
                        sc = {}
                        for h in hs:
                            sc[h] = psA.tile([128, 4, 256], F32, name="sc", tag="sc256")
                        for tl in range(4):
                            t = half * 4 + tl
                            for h in hs:
                                ct, sel = h // 2, (h % 2) * 64
                                nc.tensor.matmul(sc[h][:, tl, :],
                                                 kT[sel:sel + 64, ct, t * 128:(t + 1) * 128],
                                                 qT_bf[sel:sel + 64, ct, 256:512],
                                                 tile_position=(sel, 0),
                                                 skip_group_check=True)
                        for h in hs:
                            nc.scalar.activation(att0[h][:, half * 4:half * 4 + 4, :],
                                                 sc[h][:], AF.Exp, scale=SCALE)
                    sc1 = {}
                    for h in hs:
                        sc1[h] = psA.tile([128, 4, 256], F32, name="sc1b", tag="sc256")
                    for t in range(8, 12):
                        for h in hs:
                            ct, sel = h // 2, (h % 2) * 64
                            nc.tensor.matmul(sc1[h][:, t - 8, :],
                                             kT[sel:sel + 64, ct, t * 128:(t + 1) * 128],
                                             qT_bf[sel:sel + 64, ct, 256:512],
                                             tile_position=(sel, 0),
                                             skip_group_check=True)
                    att1 = {}
                    for h in hs:
                        a = pat.tile([128, 4, 256], BF16, tag="attB1")
                        nc.scalar.activation(a[:], sc1[h][:], AF.Exp, scale=SCALE)
                        nc.vector.tensor_tensor(a[:, :, 0:128], a[:, :, 0:128],
                                                masks[:, 1, :, 0:128], ALU.mult)
                        att1[h] = a
                    sc2 = {}
                    for h in hs:
                        sc2[h] = psA.tile([128, 4, 256], F32, name="sc2", tag="sc256")
                    for t in range(12, 16):
                        for h in hs:
                            ct, sel = h // 2, (h % 2) * 64
                            nc.tensor.matmul(sc2[h][:, t - 12, 0:128],
                                             kT[sel:sel + 64, ct, t * 128:(t + 1) * 128],
                                             qT_bf[sel:sel + 64, ct, 384:512],
                                             tile_position=(sel, 0),
                                             skip_group_check=True)
                    att2 = {}
                    for h in hs:
                        a = pat.tile([128, 4, 128], BF16, tag="attB2")
                        nc.scalar.activation(a[:], sc2[h][:, :, 0:128], AF.Exp,
                                             scale=SCALE)
                        nc.vector.tensor_tensor(a[:], a[:],
                                                masks[:, 1, :, 128:256], ALU.mult)
                        att2[h] = a
                    return {h: (att0[h], att1[h], att2[h]) for h in hs}

                def av_b(h, att0, att1, att2):
                    yp = psY.tile([65, 512], F32, tag="yp")
                    vx = lambda t: v_ext[:, t, h, :]
                    for t in range(8):
                        nc.tensor.matmul(yp[:, 0:256], vx(t), att0[:, t, :],
                                         start=(t == 0), stop=False,
                                         skip_group_check=True)
                    for t in range(8, 11):
                        nc.tensor.matmul(yp[:, 0:256], vx(t), att1[:, t - 8, :],
                                         start=False, stop=False,
                                         skip_group_check=True)
                    nc.tensor.matmul(yp[:, 0:128], vx(11), att1[:, 3, 0:128],
                                     start=False, stop=True, skip_group_check=True)
                    nc.tensor.matmul(yp[:, 128:256], vx(11), att1[:, 3, 128:256],
                                     start=False, stop=False, skip_group_check=True)
                    for t in range(12, 16):
                        nc.tensor.matmul(yp[:, 128:256], vx(t), att2[:, t - 12, :],
                                         start=False, stop=(t == 15),
                                         skip_group_check=True)
                    return yp

                def evac(h, yp, c0):
                    ct, sel = h // 2, (h % 2) * 64
                    if c0 == 0:
                        nc.scalar.copy(yT_all[sel:sel + 64, ct, c0:c0 + 256],
                                       yp[0:64, 0:256])
                    else:
                        nc.vector.tensor_copy(yT_all[sel:sel + 64, ct, c0:c0 + 256],
                                              yp[0:64, 0:256])
                    srow = pla.tile([65, 256], F32, tag="srow")
                    nc.vector.tensor_copy(srow[64:65, :], yp[64:65, 0:256])
                    if c0 == 0:
                        nc.gpsimd.dma_start(s_all[h:h + 1, 0:256],
                                            srow[64:65, :])
                    elif h < 8:
                        nc.gpsimd.dma_start(s_all[h:h + 1, 256:512],
                                            srow[64:65, :])
                    else:
                        nc.gpsimd.dma_start(s_allB2[h - 8:h - 7, :],
                                            srow[64:65, :])

                # ---------- emission: LN groups + paired-head pipeline ----------
                ln_group(q_d, range(0, 4), "q")
                ln_group(k_d, range(0, 4), "k")
                ln_group(k_d, range(4, 8), "k")
                ln_group(v_d, range(0, 4), "v")
                ln_group(v_d, range(4, 8), "v")

                apair = {}
                for j in range(9):
                    if j < 8:
                        apair[j] = qk_a_pair(2 * j)
                    if j >= 1:
                        prev = apair.pop(j - 1)
                        for hh in (2 * (j - 1), 2 * (j - 1) + 1):
                            yp = av_a(hh, *prev[hh])
                            evac(hh, yp, 0)
                    if j == 1:
                        ln_group(k_d, range(8, 16), "k")
                    if j == 2:
                        ln_group(v_d, range(8, 16), "v")

                bpair = {}
                for j in range(9):
                    if j < 8:
                        bpair[j] = qk_b_pair(2 * j)
                    if j == 1:
                        # part-A denominators: fast recip + gather while B runs
                        nc.vector.reciprocal_approx_fast(sA_rec[:],
                                                         s_all[:, 0:256])
                        nc.vector.tensor_copy(srec_b[:, 0:256], sA_rec[:])
                        for hh in range(H):
                            nc.gpsimd.dma_start(s_bf[0:1, hh * 512:hh * 512 + 256],
                                                srec_b[hh:hh + 1, 0:256])
                    if j >= 1:
                        prev = bpair.pop(j - 1)
                        for hh in (2 * (j - 1), 2 * (j - 1) + 1):
                            yp = av_b(hh, *prev[hh])
                            evac(hh, yp, 256)
                        if 2 * (j - 1) + 1 == 7:
                            nc.vector.reciprocal_approx_fast(sA_rec[0:8, :],
                                                             s_all[0:8, 256:512])
                            nc.vector.tensor_copy(srec_b[0:8, 256:512],
                                                  sA_rec[0:8, :])
                            for hh in range(8):
                                nc.gpsimd.dma_start(
                                    s_bf[0:1, hh * 512 + 256:(hh + 1) * 512],
                                    srec_b[hh:hh + 1, 256:512])
                        elif 2 * (j - 1) + 1 == 15:
                            nc.vector.reciprocal_approx_fast(sA_rec[0:8, :],
                                                             s_allB2[:])
                            nc.vector.tensor_copy(srec_b2[:], sA_rec[0:8, :])
                            for hh in range(8, 16):
                                nc.gpsimd.dma_start(
                                    s_bf[0:1, hh * 512 + 256:(hh + 1) * 512],
                                    srec_b2[hh - 8:hh - 7, :])

        # ---------- denominators + c_proj + residual -> xT ----------
        pst_cm = tc.tile_pool(name="pst", bufs=1, space="PSUM")
        pst = pst_cm.__enter__()
        s1 = pst.tile([1, 512], F32)
        s2 = pst.tile([1, 512], F32)
        with (
            tc.tile_pool(name="pcp", bufs=1) as pcp,
            tc.tile_pool(name="pcw", bufs=3) as cw,
            tc.tile_pool(name="pcr", bufs=2, space="PSUM") as csR,
            tc.tile_pool(name="pcps", bufs=1, space="PSUM") as cps,
        ):
            ysc = pcp.tile([128, NC, 512], BF16)
            for half in range(2):
                pjs = []
                for oi in range(4):
                    pjt = cps.tile([128, 512], F32, tag=f"cp{oi}")
                    pjs.append(pjt)
                for ct in range(NC):
                    if half == 0:
                        rb = csR.tile([128, 512], F32, tag="rb")
                        for hh in range(2):
                            h = ct * 2 + hh
                            for (c0, c1) in ((0, 256), (256, 512)):
                                nc.tensor.matmul(
                                    rb[hh * 64:hh * 64 + 64, c0:c1],
                                    w1_bf[0:1, h * 64:h * 64 + 64],
                                    s_bf[0:1, h * 512 + c0:h * 512 + c1],
                                    tile_position=(0, hh * 64),
                                    skip_group_check=True)
                        t1 = cw.tile([128, 512], F32, tag="yt1")
                        nc.vector.tensor_tensor(t1[:], yT_all[:, ct, :], rb[:],
                                                ALU.mult)
                        nc.vector.tensor_scalar(ysc[:, ct, :], t1[:], 1.0,
                                                ln1b(ct), ALU.mult, ALU.add)
                    for oi in range(4):
                        ot = half * 4 + oi
                        nc.tensor.matmul(
                            pjs[oi][:], cpw_sb[:, ct, ot * 128:(ot + 1) * 128],
                            ysc[:, ct, :], start=(ct == 0), stop=(ct == NC - 1))
                for oi in range(4):
                    ot = half * 4 + oi
                    t2 = cw.tile([128, 512], F32, tag="cpt")
                    nc.vector.tensor_scalar(t2[:], pjs[oi][:], 1.0, apb(ot),
                                            ALU.mult, ALU.add)
                    nc.vector.tensor_tensor(xT[:, ot, :], t2[:], qT_bf[:, ot, :],
                                            ALU.add)
                    sq = cw.tile([128, 512], BF16, tag="sq2")
                    nc.scalar.activation(sq[:], xT[:, ot, :], AF.Square)
                    nc.tensor.matmul(s1[:], ones_f[:, 0:1], xT[:, ot, :],
                                     start=(ot == 0), stop=(ot == NC - 1),
                                     skip_group_check=True)
                    nc.tensor.matmul(s2[:], ones_bf[:], sq[:],
                                     start=(ot == 0), stop=(ot == NC - 1),
                                     skip_group_check=True)

        py_cm.__exit__(None, None, None)

        # ---------- LN2 + MLP ----------
        with (
            tc.tile_pool(name="pm", bufs=1) as pm,
            tc.tile_pool(name="pmw", bufs=3) as mw,
            tc.tile_pool(name="pfw", bufs=8) as fwp,
            tc.tile_pool(name="pms", bufs=1, space="PSUM") as mps,
            tc.tile_pool(name="pma", bufs=2, space="PSUM") as mac,
        ):
            mu = pm.tile([1, 512], F32)
            nc.vector.tensor_scalar(mu[:], s1[:], 1.0 / C, None, ALU.mult)
            var = pm.tile([1, 512], F32)
            nc.vector.tensor_scalar(var[:], s2[:], 1.0 / C, EPS, ALU.mult, ALU.add)
            mu2 = pm.tile([1, 512], F32)
            nc.vector.tensor_tensor(mu2[:], mu[:], mu[:], ALU.mult)
            nc.vector.tensor_tensor(var[:], var[:], mu2[:], ALU.subtract)
            rstd2 = pm.tile([1, 512], F32)
            nc.scalar.activation(rstd2[:], var[:], AF.Ln)
            nc.scalar.activation(rstd2[:], rstd2[:], AF.Exp, scale=-0.5)
            nmr2 = pm.tile([1, 512], F32)
            nc.vector.tensor_tensor(nmr2[:], mu[:], rstd2[:], ALU.mult)
            nc.vector.tensor_scalar(nmr2[:], nmr2[:], -1.0, None, ALU.mult)
            rstd2b = pm.tile([1, 512], BF16)
            nc.vector.tensor_copy(rstd2b[:], rstd2[:])
            nmr2b = pm.tile([1, 512], BF16)
            nc.vector.tensor_copy(nmr2b[:], nmr2[:])

            zA = mps.tile([128, 512], F32, tag="zA")
            zB = mps.tile([128, 512], F32, tag="zB")
            nc.tensor.matmul(zA[:], ones_row[:], rstd2b[:], skip_group_check=True)
            nc.tensor.matmul(zB[:], ones_row[:], nmr2b[:], skip_group_check=True)

            z2 = pm.tile([128, NC, 512], BF16)
            for ct in range(NC):
                t1 = mw.tile([128, 512], F32, tag="z2t")
                nc.vector.tensor_tensor(t1[:], xT[:, ct, :], zA[:], ALU.mult)
                nc.vector.tensor_tensor(t1[:], t1[:], zB[:], ALU.add)
                nc.vector.tensor_scalar(z2[:, ct, :], t1[:],
                                        w2c(ct), b2c(ct), ALU.mult, ALU.add)

            mid = pm.tile([128, NF, 512], BF16)
            for ft in range(NF):
                fw = fwp.tile([128, NC, 128], BF16, tag="fw")
                nc.sync.dma_start(fw[:], fcw_d.ap()[ft])
                fp = mac.tile([128, 512], F32, tag="acc")
                for ct in range(NC):
                    nc.tensor.matmul(fp[:], fw[:, ct, :], z2[:, ct, :],
                                     start=(ct == 0), stop=(ct == NC - 1))
                nc.scalar.activation(mid[:, ft, :], fp[:], AF.Gelu_apprx_tanh,
                                     bias=fcb[:, ft:ft + 1])

            outT = pm.tile([128, NC, 512], F32)
            ons = [pm.tile([128, C], F32, name=f"on{i}", tag=f"on{i}") for i in range(NSLOT)]
            for ot in range(NC):
                pw = mw.tile([128, NF, 128], BF16, tag="pw")
                nc.sync.dma_start(pw[:], pjw_d.ap()[ot])
                pacc = mac.tile([128, 512], F32, tag="acc")
                for ft in range(NF):
                    nc.tensor.matmul(pacc[:], pw[:, ft, :], mid[:, ft, :],
                                     start=(ft == 0), stop=(ft == NF - 1))
                t3 = mw.tile([128, 512], F32, tag="ot3")
                nc.vector.tensor_scalar(t3[:], pacc[:], 1.0, pjb(ot),
                                        ALU.mult, ALU.add)
                nc.vector.tensor_tensor(outT[:, ot, :], t3[:], xT[:, ot, :], ALU.add)
                # out transposes ride behind proj, per-ot
                pot = mac.tile([128, 4, 128], F32, tag="po")
                for i in range(NSLOT):
                    nc.tensor.transpose(pot[:, i, :],
                                        outT[:, ot, i * 128:(i + 1) * 128],
                                        ident[:])
                for i in range(NSLOT):
                    if i % 2 == 0:
                        nc.scalar.copy(ons[i][:, ot * 128:(ot + 1) * 128],
                                       pot[:, i, :])
                    else:
                        nc.vector.tensor_copy(ons[i][:, ot * 128:(ot + 1) * 128],
                                              pot[:, i, :])
                if ot == 3:
                    for i in range(NSLOT):
                        nc.sync.dma_start(out_d.ap()[i][:, 0:512],
                                          ons[i][:, 0:512])
            for i in range(NSLOT):
                nc.sync.dma_start(out_d.ap()[i][:, 512:1024],
                                  ons[i][:, 512:1024])

        pst_cm.__exit__(None, None, None)

    nc.compile()
    return nc


def _host_prep(inputs):
    q = np.asarray(inputs["q"], np.float32)
    k = np.asarray(inputs["k"], np.float32)
    v = np.asarray(inputs["v"], np.float32)

    bf16 = ml_dtypes.bfloat16
    cpwt = np.ascontiguousarray(
        np.asarray(inputs["attn_proj_w"], np.float32).T.reshape(NC, 128, C)
        .transpose(1, 0, 2)).astype(bf16)             # [p, ct, o]
    fcwt = np.ascontiguousarray(
        np.asarray(inputs["fc_w"], np.float32).T.reshape(NC, 128, NF, 128)
        .transpose(2, 1, 0, 3)).astype(bf16)          # [ft, p, ct, f]
    pjwt = np.ascontiguousarray(
        np.asarray(inputs["proj_w"], np.float32).T.reshape(NF, 128, NC, 128)
        .transpose(2, 1, 0, 3)).astype(bf16)          # [ot, p, ft, o]

    vecs = np.ascontiguousarray(np.stack(
        [np.asarray(inputs["ln1_w"], np.float32),
         np.asarray(inputs["ln1_b"], np.float32),
         np.asarray(inputs["attn_proj_b"], np.float32),
         np.asarray(inputs["proj_b"], np.float32),
         np.asarray(inputs["ln2_w"], np.float32),
         np.asarray(inputs["ln2_b"], np.float32)], axis=1))
    w1row = np.asarray(inputs["ln1_w"], np.float32)[None, :].astype(ml_dtypes.bfloat16)
    fcb = np.ascontiguousarray(np.asarray(inputs["fc_b"], np.float32))

    in_maps, slot_map = [], []
    for c in range(N_CORES):
        b, r = c // 4, c % 4
        slots = [r, 7 - r, 8 + r, 15 - r]
        slot_map.append((b, slots))
        qs = q[b].reshape(NT, 128, C)[slots]
        # multiplicative causal masks per band chunk: keep iff
        # key (p + 512*i + 128*t) <= query (128*a_i + j).
        # mask[:, part, :, 0:128] = first slot's band,
        # mask[:, part, :, 128:256] = second slot's band.
        mask = np.ones((128, 2, 4, 256), np.float32)
        p = np.arange(128)[:, None, None]
        t = np.arange(4)[None, :, None]
        j = np.arange(128)[None, None, :]
        for part, (ba, cl) in enumerate(((0, 1), (2, 3))):
            mask[:, part, :, 0:128] = (
                (p + 512 * ba + 128 * t) <= (128 * slots[ba] + j))
            mask[:, part, :, 128:256] = (
                (p + 512 * cl + 128 * t) <= (128 * slots[cl] + j))
        in_maps.append({
            "q_s": np.ascontiguousarray(qs).astype(ml_dtypes.bfloat16),
            "k_f": np.ascontiguousarray(k[b].reshape(NT, 128, C)).astype(ml_dtypes.bfloat16),
            "v_f": np.ascontiguousarray(v[b].reshape(NT, 128, C)).astype(ml_dtypes.bfloat16),
            "mask": mask.astype(ml_dtypes.bfloat16),
            "vecs": vecs, "w1row": w1row,
            "cpwt": cpwt, "fcwt": fcwt, "pjwt": pjwt, "fcb": fcb,
        })
    return in_maps, slot_map


def kernel(**inputs):
    if "nc" not in _CACHE:
        _CACHE["nc"] = build()
    nc = _CACHE["nc"]
    in_maps, slot_map = _host_prep(inputs)
    res = run_bass_kernel_spmd(nc, in_maps, core_ids=list(range(N_CORES)))
    out = np.empty((B, T, C), np.float32)
    for c in range(N_CORES):
        b, slots = slot_map[c]
        o = res.results[c]["out"]
        for i, a in enumerate(slots):
            out[b, a * 128:(a + 1) * 128, :] = o[i]
    return out


# revision 39
# speedup vs baseline: 1.0485x; 1.0485x over previous
"""Trainium2 Bass kernel for a dense pre-LN transformer block (B=2, T=2048, C=1024, H=16).

Sharding: zero-collective sequence parallelism over 8 cores. Core c handles
batch b=c//4 and query tiles (slots) {r, 7-r, 8+r, 15-r} (r=c%4, 128 rows
each). The slot-position windows align with 4-tile causal bands: slot
position i always has its diagonal inside key tiles [4i, 4i+4), so the
program is SPMD-identical while masks are data (paged-mask offsets).

Per head, attention runs in two parts: part A covers query cols 0:256
(slots 0,1; key tiles 0..7), part B covers cols 256:512 (slots 2,3; key
tiles 0..15), so part A can start once k/v tiles 0..7 are layer-normed.
Heads are processed in PAIRS with QK matmuls interleaved between the two
heads: the pair occupies disjoint 64-row groups of the PE array, so both
the LDWEIGHTS and the MATMULs of the two heads run concurrently (~2x QK).
Causality: exact spans per chunk; only the 4-tile diagonal bands get a
bf16 mask multiply. Softmax denominators ride along as an extra
ones-column in the v stationary; their reciprocals (fast DVE approx) and
the s_bf gather are hoisted into the part-B pipeline so c_proj can start
immediately after attention. LN2 token-stat matmuls accumulate per-ot
inside c_proj. Activation tables are pinned so Exp/Ln share one set
(zero mid-kernel table swaps except one load for gelu).

Numerics: all matmuls bf16 with fp32 PSUM accumulation; residuals/LN math
fp32 (fp8 was evaluated and rejected: e4m3 matmul quantization costs
1.3-2e-2 of max-rel-err against a 2e-2 budget).
"""

import sys
import functools

sys.path.insert(0, "/opt/trn_rl_repo")

import numpy as np
import ml_dtypes

import concourse.bass as bass
import concourse.bacc as bacc
import concourse.mybir as mybir
import concourse.tile as tile
from concourse.bass_utils import run_bass_kernel_spmd

# Pin Exp/Ln to the one table set that holds both, so the act-table
# placement pass cannot thrash between exp_and_others and natural_log
# (each swap costs ~1.3us and stalls the attention softmax pipeline).
# Indices into act_info.json are preserved; only set membership as seen
# by the placement pass is narrowed.
_AF = mybir.ActivationFunctionType


@functools.cache
def _pinned_act_tables(arch):
    import concourse.hw_specs as hw_specs
    out = {}
    for name, fns in hw_specs.get_activation_tables(arch).items():
        fns = set(fns)
        if name != "natural_log_exp_and_others":
            fns.discard(_AF.Exp)
            fns.discard(_AF.Ln)
        out[name] = fns
    return out


bacc.get_activation_tables = _pinned_act_tables

F32 = mybir.dt.float32
BF16 = mybir.dt.bfloat16
FP8 = mybir.dt.float8e4
AF = mybir.ActivationFunctionType
ALU = mybir.AluOpType
DR = mybir.MatmulPerfMode.DoubleRow
WS = 64.0              # fp8 fc-weight scale

B, T, C, H, D = 2, 2048, 1024, 16, 64
NT = T // 128          # 16 key tiles
NC = C // 128          # 8 channel tiles
NF = 4 * C // 128      # 32 fc tiles
NSLOT = 4
N_CORES = 8
EPS = 1e-5
SCALE = 1.0 / 8.0      # 1/sqrt(D)

_CACHE = {}


def build():
    nc = bacc.Bacc("TRN2", target_bir_lowering=False, debug=False,
                   num_devices=N_CORES)

    q_d = nc.dram_tensor("q_s", [NSLOT, 128, C], BF16, kind="ExternalInput")
    k_d = nc.dram_tensor("k_f", [NT, 128, C], BF16, kind="ExternalInput")
    v_d = nc.dram_tensor("v_f", [NT, 128, C], BF16, kind="ExternalInput")
    mask_d = nc.dram_tensor("mask", [128, 2, 4, 256], BF16, kind="ExternalInput")
    vecs_d = nc.dram_tensor("vecs", [C, 6], F32, kind="ExternalInput")
    w1_d = nc.dram_tensor("w1row", [1, C], BF16, kind="ExternalInput")
    cpw_d = nc.dram_tensor("cpwt", [128, NC, C], BF16, kind="ExternalInput")
    fcw_d = nc.dram_tensor("fcwt", [NF, 128, NC, 128], FP8, kind="ExternalInput")
    pjw_d = nc.dram_tensor("pjwt", [NC, 128, NF, 128], BF16, kind="ExternalInput")
    fcb_d = nc.dram_tensor("fcb", [4 * C], F32, kind="ExternalInput")
    out_d = nc.dram_tensor("out", [NSLOT, 128, C], F32, kind="ExternalOutput")

    with tile.TileContext(nc) as tc:
      with tc.tile_pool(name="pg", bufs=1) as pg:
        # ---- persistent constants (gpsimd DMA queue: keeps the sync
        # queue free for the startup q/k/v tile loads) ----
        vecs = pg.tile([128, NC, 6], F32)   # ln1w ln1b apb pjb w2 b2
        nc.gpsimd.dma_start(vecs[:], vecs_d.ap().rearrange("(ct p) v -> p ct v", p=128))
        masks = pg.tile([128, 2, 4, 256], BF16)
        nc.gpsimd.dma_start(masks[:], mask_d.ap())
        w1_bf = pg.tile([1, C], BF16)
        nc.gpsimd.dma_start(w1_bf[:], w1_d.ap())
        fcb = pg.tile([128, NF], F32)
        nc.gpsimd.dma_start(fcb[:], fcb_d.ap().rearrange("(ft p) -> p ft", p=128))

        ones_f = pg.tile([128, 128], F32)
        nc.gpsimd.memset(ones_f[:], 1.0)
        ident = pg.tile([128, 128], F32)
        nc.gpsimd.affine_select(ident[:], ones_f[:], [[1, 128]], ALU.is_equal,
                                0.0, channel_multiplier=-1)
        ones128_bf = pg.tile([128, 128], BF16)
        nc.gpsimd.memset(ones128_bf[:], 1.0)
        ident_bf = pg.tile([128, 128], BF16)
        nc.gpsimd.affine_select(ident_bf[:], ones128_bf[:], [[1, 128]], ALU.is_equal,
                                0.0, channel_multiplier=-1)
        ones_bf = pg.tile([128, 1], BF16)
        nc.gpsimd.memset(ones_bf[:], 1.0)
        ones_row = pg.tile([1, 128], BF16)
        nc.gpsimd.memset(ones_row[:], 1.0)

        ln1w = lambda ct: vecs[:, ct, 0:1]
        ln1b = lambda ct: vecs[:, ct, 1:2]
        apb = lambda ct: vecs[:, ct, 2:3]
        pjb = lambda ct: vecs[:, ct, 3:4]
        w2c = lambda ct: vecs[:, ct, 4:5]
        b2c = lambda ct: vecs[:, ct, 5:6]

        # ---- cross-phase tensors ----
        qT_bf = pg.tile([128, NC, 512], BF16)   # LN1(q)^T w/ w,b (QK rhs + residual)
        xT = pg.tile([128, NC, 512], F32)       # attn residual output (C-major)

        py_cm = tc.tile_pool(name="py", bufs=1)
        py = py_cm.__enter__()
        yT_all = py.tile([128, NC, 512], F32)   # raw attention out (pre 1/s, w1, b1)
        s_all = py.tile([H, 512], F32)          # softmax denominators
        s_bf = py.tile([1, H * 512], BF16)      # reciprocals, gathered on one row
        srec_b = py.tile([H, 512], BF16)
        s_allB2 = py.tile([8, 256], F32)        # part-B denoms, heads 8..15
        srec_b2 = py.tile([8, 256], BF16)
        sA_rec = py.tile([H, 256], F32)         # fast-recip scratch (part A)
        cpw_sb = py.tile([128, NC, C], BF16)    # c_proj weights, preloaded
        nc.gpsimd.dma_start(cpw_sb[:], cpw_d.ap())

        with tc.tile_pool(name="pa", bufs=1) as pa:
            kT = pa.tile([128, NC, T], BF16)          # LN1(k)^T w/ w,b
            # v_ext cols: [v0..v63 | ones] — AV output rows 0..63 = y,
            # row 64 = softmax denominator.
            v_ext = pa.tile([128, NT, H, 65], BF16)
            for tt in range(NT):
                nc.gpsimd.memset(v_ext[:, tt, :, 64:65], 1.0)

            with (
                tc.tile_pool(name="pln", bufs=8) as pl,
                tc.tile_pool(name="plz", bufs=2) as plz,
                tc.tile_pool(name="pla", bufs=3) as pla,
                tc.tile_pool(name="pat", bufs=2) as pat,
                tc.tile_pool(name="psA", bufs=2, space="PSUM") as psA,
                tc.tile_pool(name="psB", bufs=2, space="PSUM") as psB,
                tc.tile_pool(name="psY", bufs=2, space="PSUM") as psY,
            ):
                # ---------- LN1 group: load, stats, rstd, normalize ----------
                def ln_group(src_d, tts, kind):
                    n = len(tts)
                    xs = []
                    agg = pla.tile([128, 8, 2], F32, tag="agg")
                    for gi, tt in enumerate(tts):
                        x = pl.tile([128, C], BF16, tag="xin")
                        nc.sync.dma_start(x[:], src_d.ap()[tt])
                        st6 = pl.tile([128, 2, 6], F32, tag="st6")
                        nc.vector.bn_stats(st6[:, 0, :], x[:, 0:512])
                        nc.vector.bn_stats(st6[:, 1, :], x[:, 512:1024])
                        nc.vector.bn_aggr(agg[:, gi, :], st6[:])
                        xs.append(x)
                    veps = pla.tile([128, 8], F32, tag="veps")
                    nc.vector.tensor_scalar(veps[:, 0:n], agg[:, 0:n, 1], EPS, None,
                                            ALU.add)
                    rstd = pla.tile([128, 8], F32, tag="rstd")
                    nc.scalar.activation(rstd[:, 0:n], veps[:, 0:n], AF.Ln)
                    nc.scalar.activation(rstd[:, 0:n], rstd[:, 0:n], AF.Exp,
                                         scale=-0.5)
                    nmr = pla.tile([128, 8], F32, tag="nmr")
                    nc.vector.tensor_tensor(nmr[:, 0:n], agg[:, 0:n, 0], rstd[:, 0:n],
                                            ALU.mult)
                    nc.vector.tensor_scalar(nmr[:, 0:n], nmr[:, 0:n], -1.0, None,
                                            ALU.mult)
                    if kind == "v":
                        for gi, tt in enumerate(tts):
                            nc.vector.tensor_scalar(
                                v_ext[:, tt, :, 0:64],
                                xs[gi][:].rearrange("p (h d) -> p h d", h=H),
                                rstd[:, gi:gi + 1], nmr[:, gi:gi + 1],
                                ALU.mult, ALU.add)
                        return
                    # q/k: normalize -> transpose -> evacuate with w,b
                    dstT, col0 = (qT_bf, 0) if kind == "q" else (kT, tts[0] * 128)
                    late = kind == "k" and tts[0] >= 8
                    zs = []
                    for gi, tt in enumerate(tts):
                        z = plz.tile([128, C], BF16, tag=f"z{gi % 4}")
                        nc.vector.tensor_scalar(z[:], xs[gi][:],
                                                rstd[:, gi:gi + 1], nmr[:, gi:gi + 1],
                                                ALU.mult, ALU.add)
                        zs.append(z)
                    for half in range(n // 4):
                        for ct in range(NC):
                            ps = psB.tile([128, 4, 128], F32, tag="p128")
                            pv = ps[:].bitcast(BF16)[:, :, 0:128]
                            for gi in range(4):
                                nc.tensor.transpose(
                                    pv[:, gi, :],
                                    zs[half * 4 + gi][:, ct * 128:(ct + 1) * 128],
                                    ident_bf[:])
                            dst = dstT[:, ct, col0 + half * 512:col0 + half * 512 + 512]
                            if late and ct % 2 == 0:
                                # split evacs ACT/DVE while softmax runs
                                nc.vector.tensor_scalar(dst, pv[:], ln1w(ct),
                                                        ln1b(ct), ALU.mult, ALU.add)
                            else:
                                nc.scalar.activation(dst, pv[:], AF.Identity,
                                                     bias=ln1b(ct), scale=ln1w(ct))

                # ---------- attention, head pair (h0, h0+1) ----------
                # QK matmuls interleave the two heads (disjoint 64-row PE
                # groups -> concurrent LDWEIGHTS+MATMUL for the pair).
                def qk_a_pair(h0):
                    hs = (h0, h0 + 1)
                    sc0 = {}
                    for h in hs:
                        sc0[h] = psA.tile([128, 4, 256], F32, name="sc0", tag="sc256")
                    for t in range(4):
                        for h in hs:
                            ct, sel = h // 2, (h % 2) * 64
                            nc.tensor.matmul(sc0[h][:, t, :],
                                             kT[sel:sel + 64, ct, t * 128:(t + 1) * 128],
                                             qT_bf[sel:sel + 64, ct, 0:256],
                                             tile_position=(sel, 0),
                                             skip_group_check=True)
                    att0 = {}
                    for h in hs:
                        a = pat.tile([128, 4, 256], BF16, tag="attA0")
                        nc.scalar.activation(a[:], sc0[h][:], AF.Exp, scale=SCALE)
                        nc.vector.tensor_tensor(a[:, :, 0:128], a[:, :, 0:128],
                                                masks[:, 0, :, 0:128], ALU.mult)
                        att0[h] = a
                    sc1 = {}
                    for h in hs:
                        sc1[h] = psB.tile([128, 4, 128], F32, name="sc1", tag="p128")
                    for t in range(4, 8):
                        for h in hs:
                            ct, sel = h // 2, (h % 2) * 64
                            nc.tensor.matmul(sc1[h][:, t - 4, :],
                                             kT[sel:sel + 64, ct, t * 128:(t + 1) * 128],
                                             qT_bf[sel:sel + 64, ct, 128:256],
                                             tile_position=(sel, 0),
                                             skip_group_check=True)
                    att1 = {}
                    for h in hs:
                        a = pat.tile([128, 4, 128], BF16, tag="attA1")
                        nc.scalar.activation(a[:], sc1[h][:], AF.Exp, scale=SCALE)
                        nc.vector.tensor_tensor(a[:], a[:],
                                                masks[:, 0, :, 128:256], ALU.mult)
                        att1[h] = a
                    return {h: (att0[h], att1[h]) for h in hs}

                def av_a(h, att0, att1):
                    yp = psY.tile([65, 512], F32, tag="yp")
                    vx = lambda t: v_ext[:, t, h, :]
                    for t in range(3):
                        nc.tensor.matmul(yp[:, 0:256], vx(t), att0[:, t, :],
                                         start=(t == 0), stop=False,
                                         skip_group_check=True)
                    nc.tensor.matmul(yp[:, 0:128], vx(3), att0[:, 3, 0:128],
                                     start=False, stop=True, skip_group_check=True)
                    nc.tensor.matmul(yp[:, 128:256], vx(3), att0[:, 3, 128:256],
                                     start=False, stop=False, skip_group_check=True)
                    for t in range(4, 8):
                        nc.tensor.matmul(yp[:, 128:256], vx(t), att1[:, t - 4, :],
                                         start=False, stop=(t == 7),
                                         skip_group_check=True)
                    return yp

                def qk_b_pair(h0):
                    hs = (h0, h0 + 1)
                    att0 = {h: pat.tile([128, 8, 256], BF16, name="attB0", tag="attB0") for h in hs}
                    for half in range(2):
                        sc = {}
                        for h in hs:
                            sc[h] = psA.tile([128, 4, 256], F32, name="sc", tag="sc256")
                        for tl in range(4):
                            t = half * 4 + tl
                            for h in hs:
                                ct, sel = h // 2, (h % 2) * 64
                                nc.tensor.matmul(sc[h][:, tl, :],
                                                 kT[sel:sel + 64, ct, t * 128:(t + 1) * 128],
                                                 qT_bf[sel:sel + 64, ct, 256:512],
                                                 tile_position=(sel, 0),
                                                 skip_group_check=True)
                        for h in hs:
                            nc.scalar.activation(att0[h][:, half * 4:half * 4 + 4, :],
                                                 sc[h][:], AF.Exp, scale=SCALE)
                    sc1 = {}
                    for h in hs:
                        sc1[h] = psA.tile([128, 4, 256], F32, name="sc1b", tag="sc256")
                    for t in range(8, 12):
                        for h in hs:
                            ct, sel = h // 2, (h % 2) * 64
                            nc.tensor.matmul(sc1[h][:, t - 8, :],
                                             kT[sel:sel + 64, ct, t * 128:(t + 1) * 128],
                                             qT_bf[sel:sel + 64, ct, 256:512],
                                             tile_position=(sel, 0),
                                             skip_group_check=True)
                    att1 = {}
                    for h in hs:
                        a = pat.tile([128, 4, 256], BF16, tag="attB1")
                        nc.scalar.activation(a[:], sc1[h][:], AF.Exp, scale=SCALE)
                        nc.vector.tensor_tensor(a[:, :, 0:128], a[:, :, 0:128],
                                                masks[:, 1, :, 0:128], ALU.mult)
                        att1[h] = a
                    sc2 = {}
                    for h in hs:
                        sc2[h] = psB.tile([128, 4, 128], F32, name="sc2", tag="p128")
                    for t in range(12, 16):
                        for h in hs:
                            ct, sel = h // 2, (h % 2) * 64
                            nc.tensor.matmul(sc2[h][:, t - 12, :],
                                             kT[sel:sel + 64, ct, t * 128:(t + 1) * 128],
                                             qT_bf[sel:sel + 64, ct, 384:512],
                                             tile_position=(sel, 0),
                                             skip_group_check=True)
                    att2 = {}
                    for h in hs:
                        a = pat.tile([128, 4, 128], BF16, tag="attB2")
                        nc.scalar.activation(a[:], sc2[h][:], AF.Exp, scale=SCALE)
                        nc.vector.tensor_tensor(a[:], a[:],
                                                masks[:, 1, :, 128:256], ALU.mult)
                        att2[h] = a
                    return {h: (att0[h], att1[h], att2[h]) for h in hs}

                def av_b(h, att0, att1, att2):
                    yp = psY.tile([65, 512], F32, tag="yp")
                    vx = lambda t: v_ext[:, t, h, :]
                    for t in range(8):
                        nc.tensor.matmul(yp[:, 0:256], vx(t), att0[:, t, :],
                                         start=(t == 0), stop=False,
                                         skip_group_check=True)
                    for t in range(8, 11):
                        nc.tensor.matmul(yp[:, 0:256], vx(t), att1[:, t - 8, :],
                                         start=False, stop=False,
                                         skip_group_check=True)
                    nc.tensor.matmul(yp[:, 0:128], vx(11), att1[:, 3, 0:128],
                                     start=False, stop=True, skip_group_check=True)
                    nc.tensor.matmul(yp[:, 128:256], vx(11), att1[:, 3, 128:256],
                                     start=False, stop=False, skip_group_check=True)
                    for t in range(12, 16):
                        nc.tensor.matmul(yp[:, 128:256], vx(t), att2[:, t - 12, :],
                                         start=False, stop=(t == 15),
                                         skip_group_check=True)
                    return yp

                def evac(h, yp, c0):
                    ct, sel = h // 2, (h % 2) * 64
                    if c0 == 0:
                        nc.scalar.copy(yT_all[sel:sel + 64, ct, c0:c0 + 256],
                                       yp[0:64, 0:256])
                    else:
                        nc.vector.tensor_copy(yT_all[sel:sel + 64, ct, c0:c0 + 256],
                                              yp[0:64, 0:256])
                    srow = pla.tile([65, 256], F32, tag="srow")
                    nc.vector.tensor_copy(srow[64:65, :], yp[64:65, 0:256])
                    if c0 == 0:
                        nc.gpsimd.dma_start(s_all[h:h + 1, 0:256],
                                            srow[64:65, :])
                    elif h < 8:
                        nc.gpsimd.dma_start(s_all[h:h + 1, 256:512],
                                            srow[64:65, :])
                    else:
                        nc.gpsimd.dma_start(s_allB2[h - 8:h - 7, :],
                                            srow[64:65, :])

                # ---------- emission: LN groups + paired-head pipeline ----------
                ln_group(q_d, range(0, 4), "q")
                ln_group(k_d, range(0, 4), "k")
                ln_group(k_d, range(4, 8), "k")
                ln_group(v_d, range(0, 4), "v")
                ln_group(v_d, range(4, 8), "v")

                apair = {}
                for j in range(9):
                    if j < 8:
                        apair[j] = qk_a_pair(2 * j)
                    if j >= 1:
                        prev = apair.pop(j - 1)
                        for hh in (2 * (j - 1), 2 * (j - 1) + 1):
                            yp = av_a(hh, *prev[hh])
                            evac(hh, yp, 0)
                    if j == 1:
                        ln_group(k_d, range(8, 16), "k")
                    if j == 4:
                        ln_group(v_d, range(8, 16), "v")

                bpair = {}
                for j in range(9):
                    if j < 8:
                        bpair[j] = qk_b_pair(2 * j)
                    if j == 1:
                        # part-A denominators: fast recip + gather while B runs
                        nc.vector.reciprocal_approx_fast(sA_rec[:],
                                                         s_all[:, 0:256])
                        nc.vector.tensor_copy(srec_b[:, 0:256], sA_rec[:])
                        for hh in range(H):
                            nc.gpsimd.dma_start(s_bf[0:1, hh * 512:hh * 512 + 256],
                                                srec_b[hh:hh + 1, 0:256])
                    if j >= 1:
                        prev = bpair.pop(j - 1)
                        for hh in (2 * (j - 1), 2 * (j - 1) + 1):
                            yp = av_b(hh, *prev[hh])
                            evac(hh, yp, 256)
                        if 2 * (j - 1) + 1 == 7:
                            nc.vector.reciprocal_approx_fast(sA_rec[0:8, :],
                                                             s_all[0:8, 256:512])
                            nc.vector.tensor_copy(srec_b[0:8, 256:512],
                                                  sA_rec[0:8, :])
                            for hh in range(8):
                                nc.gpsimd.dma_start(
                                    s_bf[0:1, hh * 512 + 256:(hh + 1) * 512],
                                    srec_b[hh:hh + 1, 256:512])
                        elif 2 * (j - 1) + 1 == 15:
                            nc.vector.reciprocal_approx_fast(sA_rec[0:8, :],
                                                             s_allB2[:])
                            nc.vector.tensor_copy(srec_b2[:], sA_rec[0:8, :])
                            for hh in range(8, 16):
                                nc.gpsimd.dma_start(
                                    s_bf[0:1, hh * 512 + 256:(hh + 1) * 512],
                                    srec_b2[hh - 8:hh - 7, :])

        # ---------- denominators + c_proj + residual -> xT ----------
        pst_cm = tc.tile_pool(name="pst", bufs=1, space="PSUM")
        pst = pst_cm.__enter__()
        s1 = pst.tile([1, 512], F32)
        s2 = pst.tile([1, 512], F32)
        with (
            tc.tile_pool(name="pcp", bufs=1) as pcp,
            tc.tile_pool(name="pcw", bufs=3) as cw,
            tc.tile_pool(name="pcr", bufs=2, space="PSUM") as csR,
            tc.tile_pool(name="pcps", bufs=1, space="PSUM") as cps,
        ):
            ysc = pcp.tile([128, NC, 512], BF16)
            for half in range(2):
                pjs = []
                for oi in range(4):
                    pjt = cps.tile([128, 512], F32, tag=f"cp{oi}")
                    pjs.append(pjt)
                for ct in range(NC):
                    if half == 0:
                        rb = csR.tile([128, 512], F32, tag="rb")
                        for hh in range(2):
                            h = ct * 2 + hh
                            for (c0, c1) in ((0, 256), (256, 512)):
                                nc.tensor.matmul(
                                    rb[hh * 64:hh * 64 + 64, c0:c1],
                                    w1_bf[0:1, h * 64:h * 64 + 64],
                                    s_bf[0:1, h * 512 + c0:h * 512 + c1],
                                    tile_position=(0, hh * 64),
                                    skip_group_check=True)
                        t1 = cw.tile([128, 512], F32, tag="yt1")
                        nc.vector.tensor_tensor(t1[:], yT_all[:, ct, :], rb[:],
                                                ALU.mult)
                        nc.vector.tensor_scalar(ysc[:, ct, :], t1[:], 1.0,
                                                ln1b(ct), ALU.mult, ALU.add)
                    for oi in range(4):
                        ot = half * 4 + oi
                        nc.tensor.matmul(
                            pjs[oi][:], cpw_sb[:, ct, ot * 128:(ot + 1) * 128],
                            ysc[:, ct, :], start=(ct == 0), stop=(ct == NC - 1))
                for oi in range(4):
                    ot = half * 4 + oi
                    t2 = cw.tile([128, 512], F32, tag="cpt")
                    nc.vector.tensor_scalar(t2[:], pjs[oi][:], 1.0, apb(ot),
                                            ALU.mult, ALU.add)
                    nc.vector.tensor_tensor(xT[:, ot, :], t2[:], qT_bf[:, ot, :],
                                            ALU.add)
                    sq = cw.tile([128, 512], BF16, tag="sq2")
                    nc.scalar.activation(sq[:], xT[:, ot, :], AF.Square)
                    nc.tensor.matmul(s1[:], ones_f[:, 0:1], xT[:, ot, :],
                                     start=(ot == 0), stop=(ot == NC - 1),
                                     skip_group_check=True)
                    nc.tensor.matmul(s2[:], ones_bf[:], sq[:],
                                     start=(ot == 0), stop=(ot == NC - 1),
                                     skip_group_check=True)

        py_cm.__exit__(None, None, None)

        # ---------- LN2 + MLP ----------
        with (
            tc.tile_pool(name="pm", bufs=1) as pm,
            tc.tile_pool(name="pmw", bufs=3) as mw,
            tc.tile_pool(name="pfw", bufs=8) as fwp,
            tc.tile_pool(name="pms", bufs=1, space="PSUM") as mps,
            tc.tile_pool(name="pma", bufs=2, space="PSUM") as mac,
        ):
            mu = pm.tile([1, 512], F32)
            nc.vector.tensor_scalar(mu[:], s1[:], 1.0 / C, None, ALU.mult)
            var = pm.tile([1, 512], F32)
            nc.vector.tensor_scalar(var[:], s2[:], 1.0 / C, EPS, ALU.mult, ALU.add)
            mu2 = pm.tile([1, 512], F32)
            nc.vector.tensor_tensor(mu2[:], mu[:], mu[:], ALU.mult)
            nc.vector.tensor_tensor(var[:], var[:], mu2[:], ALU.subtract)
            rstd2 = pm.tile([1, 512], F32)
            nc.scalar.activation(rstd2[:], var[:], AF.Ln)
            nc.scalar.activation(rstd2[:], rstd2[:], AF.Exp, scale=-0.5)
            nmr2 = pm.tile([1, 512], F32)
            nc.vector.tensor_tensor(nmr2[:], mu[:], rstd2[:], ALU.mult)
            nc.vector.tensor_scalar(nmr2[:], nmr2[:], -1.0, None, ALU.mult)
            rstd2b = pm.tile([1, 512], BF16)
            nc.vector.tensor_copy(rstd2b[:], rstd2[:])
            nmr2b = pm.tile([1, 512], BF16)
            nc.vector.tensor_copy(nmr2b[:], nmr2[:])

            zA = mps.tile([128, 512], F32, tag="zA")
            zB = mps.tile([128, 512], F32, tag="zB")
            nc.tensor.matmul(zA[:], ones_row[:], rstd2b[:], skip_group_check=True)
            nc.tensor.matmul(zB[:], ones_row[:], nmr2b[:], skip_group_check=True)

            z2 = pm.tile([128, NC, 512], FP8)
            for ct in range(NC):
                t1 = mw.tile([128, 512], F32, tag="z2t")
                nc.vector.tensor_tensor(t1[:], xT[:, ct, :], zA[:], ALU.mult)
                nc.vector.tensor_tensor(t1[:], t1[:], zB[:], ALU.add)
                nc.vector.tensor_scalar(z2[:, ct, :], t1[:],
                                        w2c(ct), b2c(ct), ALU.mult, ALU.add)

            mid = pm.tile([128, NF, 512], BF16)
            for ft in range(NF):
                fw = fwp.tile([128, NC, 128], FP8, tag="fw")
                nc.sync.dma_start(fw[:], fcw_d.ap()[ft])
                fp = mac.tile([128, 512], F32, tag="acc")
                for cp in range(NC // 2):
                    nc.tensor.matmul(fp[:], fw[:, 2 * cp:2 * cp + 2, :],
                                     z2[:, 2 * cp:2 * cp + 2, :],
                                     start=(cp == 0), stop=(cp == NC // 2 - 1),
                                     perf_mode=DR)
                nc.scalar.activation(mid[:, ft, :], fp[:], AF.Gelu_apprx_tanh,
                                     bias=fcb[:, ft:ft + 1], scale=1.0 / WS)

            outT = pm.tile([128, NC, 512], F32)
            ons = [pm.tile([128, C], F32, name=f"on{i}", tag=f"on{i}") for i in range(NSLOT)]
            for ot in range(NC):
                pw = mw.tile([128, NF, 128], BF16, tag="pw")
                nc.sync.dma_start(pw[:], pjw_d.ap()[ot])
                pacc = mac.tile([128, 512], F32, tag="acc")
                for ft in range(NF):
                    nc.tensor.matmul(pacc[:], pw[:, ft, :], mid[:, ft, :],
                                     start=(ft == 0), stop=(ft == NF - 1))
                t3 = mw.tile([128, 512], F32, tag="ot3")
                nc.vector.tensor_scalar(t3[:], pacc[:], 1.0, pjb(ot),
                                        ALU.mult, ALU.add)
                nc.vector.tensor_tensor(outT[:, ot, :], t3[:], xT[:, ot, :], ALU.add)
                # out transposes ride behind proj, per-ot
                pot = mac.tile([128, 4, 128], F32, tag="po")
                for i in range(NSLOT):
                    nc.tensor.transpose(pot[:, i, :],
                                        outT[:, ot, i * 128:(i + 1) * 128],
                                        ident[:])
                for i in range(NSLOT):
                    if i % 2 == 0:
                        nc.scalar.copy(ons[i][:, ot * 128:(ot + 1) * 128],
                                       pot[:, i, :])
                    else:
                        nc.vector.tensor_copy(ons[i][:, ot * 128:(ot + 1) * 128],
                                              pot[:, i, :])
                if ot == 3:
                    for i in range(NSLOT):
                        nc.sync.dma_start(out_d.ap()[i][:, 0:512],
                                          ons[i][:, 0:512])
            for i in range(NSLOT):
                nc.sync.dma_start(out_d.ap()[i][:, 512:1024],
                                  ons[i][:, 512:1024])

        pst_cm.__exit__(None, None, None)

    nc.compile()
    return nc


def _host_prep(inputs):
    q = np.asarray(inputs["q"], np.float32)
    k = np.asarray(inputs["k"], np.float32)
    v = np.asarray(inputs["v"], np.float32)

    bf16 = ml_dtypes.bfloat16
    cpwt = np.ascontiguousarray(
        np.asarray(inputs["attn_proj_w"], np.float32).T.reshape(NC, 128, C)
        .transpose(1, 0, 2)).astype(bf16)             # [p, ct, o]
    fcwt = np.ascontiguousarray(
        np.asarray(inputs["fc_w"], np.float32).T.reshape(NC, 128, NF, 128)
        .transpose(2, 1, 0, 3) * WS).astype(ml_dtypes.float8_e4m3)  # [ft, p, ct, f]
    pjwt = np.ascontiguousarray(
        np.asarray(inputs["proj_w"], np.float32).T.reshape(NF, 128, NC, 128)
        .transpose(2, 1, 0, 3)).astype(bf16)          # [ot, p, ft, o]

    vecs = np.ascontiguousarray(np.stack(
        [np.asarray(inputs["ln1_w"], np.float32),
         np.asarray(inputs["ln1_b"], np.float32),
         np.asarray(inputs["attn_proj_b"], np.float32),
         np.asarray(inputs["proj_b"], np.float32),
         np.asarray(inputs["ln2_w"], np.float32),
         np.asarray(inputs["ln2_b"], np.float32)], axis=1))
    w1row = np.asarray(inputs["ln1_w"], np.float32)[None, :].astype(ml_dtypes.bfloat16)
    fcb = np.ascontiguousarray(np.asarray(inputs["fc_b"], np.float32))

    in_maps, slot_map = [], []
    for c in range(N_CORES):
        b, r = c // 4, c % 4
        slots = [r, 7 - r, 8 + r, 15 - r]
        slot_map.append((b, slots))
        qs = q[b].reshape(NT, 128, C)[slots]
        # multiplicative causal masks per band chunk: keep iff
        # key (p + 512*i + 128*t) <= query (128*a_i + j).
        # mask[:, part, :, 0:128] = first slot's band,
        # mask[:, part, :, 128:256] = second slot's band.
        mask = np.ones((128, 2, 4, 256), np.float32)
        p = np.arange(128)[:, None, None]
        t = np.arange(4)[None, :, None]
        j = np.arange(128)[None, None, :]
        for part, (ba, cl) in enumerate(((0, 1), (2, 3))):
            mask[:, part, :, 0:128] = (
                (p + 512 * ba + 128 * t) <= (128 * slots[ba] + j))
            mask[:, part, :, 128:256] = (
                (p + 512 * cl + 128 * t) <= (128 * slots[cl] + j))
        in_maps.append({
            "q_s": np.ascontiguousarray(qs).astype(ml_dtypes.bfloat16),
            "k_f": np.ascontiguousarray(k[b].reshape(NT, 128, C)).astype(ml_dtypes.bfloat16),
            "v_f": np.ascontiguousarray(v[b].reshape(NT, 128, C)).astype(ml_dtypes.bfloat16),
            "mask": mask.astype(ml_dtypes.bfloat16),
            "vecs": vecs, "w1row": w1row,
            "cpwt": cpwt, "fcwt": fcwt, "pjwt": pjwt, "fcb": fcb,
        })
    return in_maps, slot_map


def kernel(**inputs):
    if "nc" not in _CACHE:
        _CACHE["nc"] = build()
    nc = _CACHE["nc"]
    in_maps, slot_map = _host_prep(inputs)
    res = run_bass_kernel_spmd(nc, in_maps, core_ids=list(range(N_CORES)))
    out = np.empty((B, T, C), np.float32)
    for c in range(N_CORES):
        b, slots = slot_map[c]
        o = res.results[c]["out"]
        for i, a in enumerate(slots):
            out[b, a * 128:(a + 1) * 128, :] = o[i]
    return out


# revision 43
# speedup vs baseline: 1.0770x; 1.0272x over previous
"""Trainium2 Bass kernel for a dense pre-LN transformer block (B=2, T=2048, C=1024, H=16).

Sharding: zero-collective sequence parallelism over 8 cores. Core c handles
batch b=c//4 and query tiles (slots) {r, 7-r, 8+r, 15-r} (r=c%4, 128 rows
each). The slot-position windows align with 4-tile causal bands: slot
position i always has its diagonal inside key tiles [4i, 4i+4), so the
program is SPMD-identical while masks are data (paged-mask offsets).

Per head, attention runs in two parts: part A covers query cols 0:256
(slots 0,1; key tiles 0..7), part B covers cols 256:512 (slots 2,3; key
tiles 0..15), so part A can start once k/v tiles 0..7 are layer-normed.
Heads are processed in PAIRS with QK matmuls interleaved between the two
heads: the pair occupies disjoint 64-row groups of the PE array, so both
the LDWEIGHTS and the MATMULs of the two heads run concurrently (~2x QK).
Causality: exact spans per chunk; only the 4-tile diagonal bands get a
bf16 mask multiply. Softmax denominators ride along as an extra
ones-column in the v stationary; their reciprocals (fast DVE approx) and
the s_bf gather are hoisted into the part-B pipeline so c_proj can start
immediately after attention. LN2 token-stat matmuls accumulate per-ot
inside c_proj. Activation tables are pinned so Exp/Ln share one set
(zero mid-kernel table swaps except one load for gelu).

Numerics: all matmuls bf16 with fp32 PSUM accumulation; residuals/LN math
fp32 (fp8 was evaluated and rejected: e4m3 matmul quantization costs
1.3-2e-2 of max-rel-err against a 2e-2 budget).
"""

import sys
import functools

sys.path.insert(0, "/opt/trn_rl_repo")

import numpy as np
import ml_dtypes

import concourse.bass as bass
import concourse.bacc as bacc
import concourse.mybir as mybir
import concourse.tile as tile
from concourse.bass_utils import run_bass_kernel_spmd

# Pin Exp/Ln to the one table set that holds both, so the act-table
# placement pass cannot thrash between exp_and_others and natural_log
# (each swap costs ~1.3us and stalls the attention softmax pipeline).
# Indices into act_info.json are preserved; only set membership as seen
# by the placement pass is narrowed.
_AF = mybir.ActivationFunctionType


@functools.cache
def _pinned_act_tables(arch):
    import concourse.hw_specs as hw_specs
    out = {}
    for name, fns in hw_specs.get_activation_tables(arch).items():
        fns = set(fns)
        if name != "natural_log_exp_and_others":
            fns.discard(_AF.Exp)
            fns.discard(_AF.Ln)
        out[name] = fns
    return out


bacc.get_activation_tables = _pinned_act_tables

F32 = mybir.dt.float32
BF16 = mybir.dt.bfloat16
AF = mybir.ActivationFunctionType
ALU = mybir.AluOpType

B, T, C, H, D = 2, 2048, 1024, 16, 64
NT = T // 128          # 16 key tiles
NC = C // 128          # 8 channel tiles
NF = 4 * C // 128      # 32 fc tiles
NSLOT = 4
N_CORES = 8
EPS = 1e-5
SCALE = 1.0 / 8.0      # 1/sqrt(D)

_CACHE = {}


def build():
    nc = bacc.Bacc("TRN2", target_bir_lowering=False, debug=False,
                   num_devices=N_CORES)

    q_d = nc.dram_tensor("q_s", [NSLOT, 128, C], BF16, kind="ExternalInput")
    k_d = nc.dram_tensor("k_f", [NT, 128, C], BF16, kind="ExternalInput")
    v_d = nc.dram_tensor("v_f", [NT, 128, C], BF16, kind="ExternalInput")
    mask_d = nc.dram_tensor("mask", [128, 2, 4, 256], BF16, kind="ExternalInput")
    vecs_d = nc.dram_tensor("vecs", [C, 6], F32, kind="ExternalInput")
    w1_d = nc.dram_tensor("w1row", [1, C], BF16, kind="ExternalInput")
    cpw_d = nc.dram_tensor("cpwt", [128, NC, C], BF16, kind="ExternalInput")
    fcw_d = nc.dram_tensor("fcwt", [NF, 128, NC, 128], BF16, kind="ExternalInput")
    pjw_d = nc.dram_tensor("pjwt", [NC, 128, NF, 128], BF16, kind="ExternalInput")
    fcb_d = nc.dram_tensor("fcb", [4 * C], F32, kind="ExternalInput")
    out_d = nc.dram_tensor("out", [NSLOT, 128, C], F32, kind="ExternalOutput")

    with tile.TileContext(nc) as tc:
      with tc.tile_pool(name="pg", bufs=1) as pg:
        # ---- persistent constants (gpsimd DMA queue: keeps the sync
        # queue free for the startup q/k/v tile loads) ----
        vecs = pg.tile([128, NC, 6], F32)   # ln1w ln1b apb pjb w2 b2
        nc.gpsimd.dma_start(vecs[:], vecs_d.ap().rearrange("(ct p) v -> p ct v", p=128))
        masks = pg.tile([128, 2, 4, 256], BF16)
        nc.gpsimd.dma_start(masks[:], mask_d.ap())
        w1_bf = pg.tile([1, C], BF16)
        nc.gpsimd.dma_start(w1_bf[:], w1_d.ap())
        fcb = pg.tile([128, NF], F32)
        nc.gpsimd.dma_start(fcb[:], fcb_d.ap().rearrange("(ft p) -> p ft", p=128))

        ones_f = pg.tile([128, 128], F32)
        nc.gpsimd.memset(ones_f[:], 1.0)
        ident = pg.tile([128, 128], F32)
        nc.gpsimd.affine_select(ident[:], ones_f[:], [[1, 128]], ALU.is_equal,
                                0.0, channel_multiplier=-1)
        ones128_bf = pg.tile([128, 128], BF16)
        nc.gpsimd.memset(ones128_bf[:], 1.0)
        ident_bf = pg.tile([128, 128], BF16)
        nc.gpsimd.affine_select(ident_bf[:], ones128_bf[:], [[1, 128]], ALU.is_equal,
                                0.0, channel_multiplier=-1)
        ones_bf = pg.tile([128, 1], BF16)
        nc.gpsimd.memset(ones_bf[:], 1.0)
        ones_row = pg.tile([1, 128], BF16)
        nc.gpsimd.memset(ones_row[:], 1.0)

        ln1w = lambda ct: vecs[:, ct, 0:1]
        ln1b = lambda ct: vecs[:, ct, 1:2]
        apb = lambda ct: vecs[:, ct, 2:3]
        pjb = lambda ct: vecs[:, ct, 3:4]
        w2c = lambda ct: vecs[:, ct, 4:5]
        b2c = lambda ct: vecs[:, ct, 5:6]

        # ---- cross-phase tensors ----
        qT_bf = pg.tile([128, NC, 512], BF16)   # LN1(q)^T w/ w,b (QK rhs + residual)
        xT = pg.tile([128, NC, 512], F32)       # attn residual output (C-major)

        py_cm = tc.tile_pool(name="py", bufs=1)
        py = py_cm.__enter__()
        yT_all = py.tile([128, NC, 512], F32)   # raw attention out (pre 1/s, w1, b1)
        s_all = py.tile([H, 512], F32)          # softmax denominators
        s_bf = py.tile([1, H * 512], BF16)      # reciprocals, gathered on one row
        srec_b = py.tile([H, 512], BF16)
        s_allB2 = py.tile([8, 256], F32)        # part-B denoms, heads 8..15
        srec_b2 = py.tile([8, 256], BF16)
        sA_rec = py.tile([H, 256], F32)         # fast-recip scratch (part A)
        cpw_sb = py.tile([128, NC, C], BF16)    # c_proj weights, preloaded
        nc.gpsimd.dma_start(cpw_sb[:], cpw_d.ap())

        with tc.tile_pool(name="pa", bufs=1) as pa:
            kT = pa.tile([128, NC, T], BF16)          # LN1(k)^T w/ w,b
            # v_ext cols: [v0..v63 | ones] — AV output rows 0..63 = y,
            # row 64 = softmax denominator.
            v_ext = pa.tile([128, NT, H, 65], BF16)
            for tt in range(NT):
                nc.gpsimd.memset(v_ext[:, tt, :, 64:65], 1.0)

            with (
                tc.tile_pool(name="plx", bufs=2) as plx,
                tc.tile_pool(name="pln", bufs=8) as pl,
                tc.tile_pool(name="plz", bufs=2) as plz,
                tc.tile_pool(name="pla", bufs=3) as pla,
                tc.tile_pool(name="pat", bufs=2) as pat,
                tc.tile_pool(name="psA", bufs=2, space="PSUM") as psA,
                tc.tile_pool(name="psB", bufs=2, space="PSUM") as psB,
                tc.tile_pool(name="psY", bufs=2, space="PSUM") as psY,
            ):
                # ---------- LN1 group: load, stats, rstd, normalize ----------
                def ln_group(src_d, tts, kind):
                    n = len(tts)
                    xs = []
                    agg = pla.tile([128, 8, 2], F32, tag="agg")
                    for g4 in range(0, n, 4):
                        x4 = plx.tile([128, 4, C], BF16, name="x4", tag="x4")
                        nc.sync.dma_start(
                            x4[:],
                            src_d.ap()[tts[g4]:tts[g4] + 4]
                            .rearrange("t p c -> p t c"))
                        for s in range(4):
                            xs.append((x4, s))
                    for gi in range(n):
                        x4, s = xs[gi]
                        st6 = pl.tile([128, 2, 6], F32, tag="st6")
                        nc.vector.bn_stats(st6[:, 0, :], x4[:, s, 0:512])
                        nc.vector.bn_stats(st6[:, 1, :], x4[:, s, 512:1024])
                        nc.vector.bn_aggr(agg[:, gi, :], st6[:])
                    veps = pla.tile([128, 8], F32, tag="veps")
                    nc.vector.tensor_scalar(veps[:, 0:n], agg[:, 0:n, 1], EPS, None,
                                            ALU.add)
                    rstd = pla.tile([128, 8], F32, tag="rstd")
                    nc.scalar.activation(rstd[:, 0:n], veps[:, 0:n], AF.Ln)
                    nc.scalar.activation(rstd[:, 0:n], rstd[:, 0:n], AF.Exp,
                                         scale=-0.5)
                    nmr = pla.tile([128, 8], F32, tag="nmr")
                    nc.vector.tensor_tensor(nmr[:, 0:n], agg[:, 0:n, 0], rstd[:, 0:n],
                                            ALU.mult)
                    nc.vector.tensor_scalar(nmr[:, 0:n], nmr[:, 0:n], -1.0, None,
                                            ALU.mult)
                    if kind == "v":
                        for gi, tt in enumerate(tts):
                            x4, s = xs[gi]
                            nc.vector.tensor_scalar(
                                v_ext[:, tt, :, 0:64],
                                x4[:, s, :].rearrange("p (h d) -> p h d", h=H),
                                rstd[:, gi:gi + 1], nmr[:, gi:gi + 1],
                                ALU.mult, ALU.add)
                        return
                    # q/k: normalize -> transpose -> evacuate with w,b
                    dstT, col0 = (qT_bf, 0) if kind == "q" else (kT, tts[0] * 128)
                    late = kind == "k" and tts[0] >= 8
                    zs = []
                    for gi, tt in enumerate(tts):
                        x4, s = xs[gi]
                        z = plz.tile([128, C], BF16, tag=f"z{gi % 4}")
                        nc.vector.tensor_scalar(z[:], x4[:, s, :],
                                                rstd[:, gi:gi + 1], nmr[:, gi:gi + 1],
                                                ALU.mult, ALU.add)
                        zs.append(z)
                    for half in range(n // 4):
                        for ct in range(NC):
                            ps = psB.tile([128, 4, 128], F32, tag="p128")
                            pv = ps[:].bitcast(BF16)[:, :, 0:128]
                            for gi in range(4):
                                nc.tensor.transpose(
                                    pv[:, gi, :],
                                    zs[half * 4 + gi][:, ct * 128:(ct + 1) * 128],
                                    ident_bf[:])
                            dst = dstT[:, ct, col0 + half * 512:col0 + half * 512 + 512]
                            if late and ct % 2 == 0:
                                # split evacs ACT/DVE while softmax runs
                                nc.vector.tensor_scalar(dst, pv[:], ln1w(ct),
                                                        ln1b(ct), ALU.mult, ALU.add)
                            else:
                                nc.scalar.activation(dst, pv[:], AF.Identity,
                                                     bias=ln1b(ct), scale=ln1w(ct))

                # ---------- attention, head pair (h0, h0+1) ----------
                # QK matmuls interleave the two heads (disjoint 64-row PE
                # groups -> concurrent LDWEIGHTS+MATMUL for the pair).
                def qk_a_pair(h0):
                    hs = (h0, h0 + 1)
                    sc0 = {}
                    for h in hs:
                        sc0[h] = psA.tile([128, 4, 256], F32, name="sc0", tag="sc256")
                    for t in range(4):
                        for h in hs:
                            ct, sel = h // 2, (h % 2) * 64
                            nc.tensor.matmul(sc0[h][:, t, :],
                                             kT[sel:sel + 64, ct, t * 128:(t + 1) * 128],
                                             qT_bf[sel:sel + 64, ct, 0:256],
                                             tile_position=(sel, 0),
                                             skip_group_check=True)
                    att0 = {}
                    for h in hs:
                        a = pat.tile([128, 4, 256], BF16, tag="attA0")
                        nc.scalar.activation(a[:], sc0[h][:], AF.Exp, scale=SCALE)
                        nc.vector.tensor_tensor(a[:, :, 0:128], a[:, :, 0:128],
                                                masks[:, 0, :, 0:128], ALU.mult)
                        att0[h] = a
                    sc1 = {}
                    for h in hs:
                        sc1[h] = psB.tile([128, 4, 128], F32, name="sc1", tag="p128")
                    for t in range(4, 8):
                        for h in hs:
                            ct, sel = h // 2, (h % 2) * 64
                            nc.tensor.matmul(sc1[h][:, t - 4, :],
                                             kT[sel:sel + 64, ct, t * 128:(t + 1) * 128],
                                             qT_bf[sel:sel + 64, ct, 128:256],
                                             tile_position=(sel, 0),
                                             skip_group_check=True)
                    att1 = {}
                    for h in hs:
                        a = pat.tile([128, 4, 128], BF16, tag="attA1")
                        nc.scalar.activation(a[:], sc1[h][:], AF.Exp, scale=SCALE)
                        nc.vector.tensor_tensor(a[:], a[:],
                                                masks[:, 0, :, 128:256], ALU.mult)
                        att1[h] = a
                    return {h: (att0[h], att1[h]) for h in hs}

                def av_a(h, att0, att1):
                    yp = psY.tile([65, 512], F32, tag="yp")
                    vx = lambda t: v_ext[:, t, h, :]
                    for t in range(3):
                        nc.tensor.matmul(yp[:, 0:256], vx(t), att0[:, t, :],
                                         start=(t == 0), stop=False,
                                         skip_group_check=True)
                    nc.tensor.matmul(yp[:, 0:128], vx(3), att0[:, 3, 0:128],
                                     start=False, stop=True, skip_group_check=True)
                    nc.tensor.matmul(yp[:, 128:256], vx(3), att0[:, 3, 128:256],
                                     start=False, stop=False, skip_group_check=True)
                    for t in range(4, 8):
                        nc.tensor.matmul(yp[:, 128:256], vx(t), att1[:, t - 4, :],
                                         start=False, stop=(t == 7),
                                         skip_group_check=True)
                    return yp

                def qk_b_pair(h0):
                    hs = (h0, h0 + 1)
                    att0 = {h: pat.tile([128, 8, 256], BF16, name="attB0", tag="attB0") for h in hs}
                    for half in range(2):
                        sc = {}
                        for h in hs:
                            sc[h] = psA.tile([128, 4, 256], F32, name="sc", tag="sc256")
                        for tl in range(4):
                            t = half * 4 + tl
                            for h in hs:
                                ct, sel = h // 2, (h % 2) * 64
                                nc.tensor.matmul(sc[h][:, tl, :],
                                                 kT[sel:sel + 64, ct, t * 128:(t + 1) * 128],
                                                 qT_bf[sel:sel + 64, ct, 256:512],
                                                 tile_position=(sel, 0),
                                                 skip_group_check=True)
                        for h in hs:
                            nc.scalar.activation(att0[h][:, half * 4:half * 4 + 4, :],
                                                 sc[h][:], AF.Exp, scale=SCALE)
                    sc1 = {}
                    for h in hs:
                        sc1[h] = psA.tile([128, 4, 256], F32, name="sc1b", tag="sc256")
                    for t in range(8, 12):
                        for h in hs:
                            ct, sel = h // 2, (h % 2) * 64
                            nc.tensor.matmul(sc1[h][:, t - 8, :],
                                             kT[sel:sel + 64, ct, t * 128:(t + 1) * 128],
                                             qT_bf[sel:sel + 64, ct, 256:512],
                                             tile_position=(sel, 0),
                                             skip_group_check=True)
                    att1 = {}
                    for h in hs:
                        a = pat.tile([128, 4, 256], BF16, tag="attB1")
                        nc.scalar.activation(a[:], sc1[h][:], AF.Exp, scale=SCALE)
                        nc.vector.tensor_tensor(a[:, :, 0:128], a[:, :, 0:128],
                                                masks[:, 1, :, 0:128], ALU.mult)
                        att1[h] = a
                    sc2 = {}
                    for h in hs:
                        sc2[h] = psB.tile([128, 4, 128], F32, name="sc2", tag="p128")
                    for t in range(12, 16):
                        for h in hs:
                            ct, sel = h // 2, (h % 2) * 64
                            nc.tensor.matmul(sc2[h][:, t - 12, :],
                                             kT[sel:sel + 64, ct, t * 128:(t + 1) * 128],
                                             qT_bf[sel:sel + 64, ct, 384:512],
                                             tile_position=(sel, 0),
                                             skip_group_check=True)
                    att2 = {}
                    for h in hs:
                        a = pat.tile([128, 4, 128], BF16, tag="attB2")
                        nc.scalar.activation(a[:], sc2[h][:], AF.Exp, scale=SCALE)
                        nc.vector.tensor_tensor(a[:], a[:],
                                                masks[:, 1, :, 128:256], ALU.mult)
                        att2[h] = a
                    return {h: (att0[h], att1[h], att2[h]) for h in hs}

                def av_b(h, att0, att1, att2):
                    yp = psY.tile([65, 512], F32, tag="yp")
                    vx = lambda t: v_ext[:, t, h, :]
                    for t in range(8):
                        nc.tensor.matmul(yp[:, 0:256], vx(t), att0[:, t, :],
                                         start=(t == 0), stop=False,
                                         skip_group_check=True)
                    for t in range(8, 11):
                        nc.tensor.matmul(yp[:, 0:256], vx(t), att1[:, t - 8, :],
                                         start=False, stop=False,
                                         skip_group_check=True)
                    nc.tensor.matmul(yp[:, 0:128], vx(11), att1[:, 3, 0:128],
                                     start=False, stop=True, skip_group_check=True)
                    nc.tensor.matmul(yp[:, 128:256], vx(11), att1[:, 3, 128:256],
                                     start=False, stop=False, skip_group_check=True)
                    for t in range(12, 16):
                        nc.tensor.matmul(yp[:, 128:256], vx(t), att2[:, t - 12, :],
                                         start=False, stop=(t == 15),
                                         skip_group_check=True)
                    return yp

                def evac(h, yp, c0):
                    ct, sel = h // 2, (h % 2) * 64
                    if c0 == 0:
                        nc.scalar.copy(yT_all[sel:sel + 64, ct, c0:c0 + 256],
                                       yp[0:64, 0:256])
                    else:
                        nc.vector.tensor_copy(yT_all[sel:sel + 64, ct, c0:c0 + 256],
                                              yp[0:64, 0:256])
                    srow = pla.tile([65, 256], F32, tag="srow")
                    nc.vector.tensor_copy(srow[64:65, :], yp[64:65, 0:256])
                    if c0 == 0:
                        nc.gpsimd.dma_start(s_all[h:h + 1, 0:256],
                                            srow[64:65, :])
                    elif h < 8:
                        nc.gpsimd.dma_start(s_all[h:h + 1, 256:512],
                                            srow[64:65, :])
                    else:
                        nc.gpsimd.dma_start(s_allB2[h - 8:h - 7, :],
                                            srow[64:65, :])

                # ---------- emission: LN groups + paired-head pipeline ----------
                ln_group(q_d, range(0, 4), "q")
                ln_group(k_d, range(0, 4), "k")
                ln_group(k_d, range(4, 8), "k")
                ln_group(v_d, range(0, 4), "v")
                ln_group(v_d, range(4, 8), "v")

                apair = {}
                for j in range(9):
                    if j < 8:
                        apair[j] = qk_a_pair(2 * j)
                    if j >= 1:
                        prev = apair.pop(j - 1)
                        for hh in (2 * (j - 1), 2 * (j - 1) + 1):
                            yp = av_a(hh, *prev[hh])
                            evac(hh, yp, 0)
                    if j == 1:
                        ln_group(k_d, range(8, 16), "k")
                    if j == 4:
                        ln_group(v_d, range(8, 16), "v")

                bpair = {}
                for j in range(9):
                    if j < 8:
                        bpair[j] = qk_b_pair(2 * j)
                    if j == 1:
                        # part-A denominators: fast recip + gather while B runs
                        nc.vector.reciprocal_approx_fast(sA_rec[:],
                                                         s_all[:, 0:256])
                        nc.vector.tensor_copy(srec_b[:, 0:256], sA_rec[:])
                        for hh in range(H):
                            nc.gpsimd.dma_start(s_bf[0:1, hh * 512:hh * 512 + 256],
                                                srec_b[hh:hh + 1, 0:256])
                    if j >= 1:
                        prev = bpair.pop(j - 1)
                        for hh in (2 * (j - 1), 2 * (j - 1) + 1):
                            yp = av_b(hh, *prev[hh])
                            evac(hh, yp, 256)
                        if 2 * (j - 1) + 1 == 7:
                            nc.vector.reciprocal_approx_fast(sA_rec[0:8, :],
                                                             s_all[0:8, 256:512])
                            nc.vector.tensor_copy(srec_b[0:8, 256:512],
                                                  sA_rec[0:8, :])
                            for hh in range(8):
                                nc.gpsimd.dma_start(
                                    s_bf[0:1, hh * 512 + 256:(hh + 1) * 512],
                                    srec_b[hh:hh + 1, 256:512])
                        elif 2 * (j - 1) + 1 == 15:
                            nc.vector.reciprocal_approx_fast(sA_rec[0:8, :],
                                                             s_allB2[:])
                            nc.vector.tensor_copy(srec_b2[:], sA_rec[0:8, :])
                            for hh in range(8, 16):
                                nc.gpsimd.dma_start(
                                    s_bf[0:1, hh * 512 + 256:(hh + 1) * 512],
                                    srec_b2[hh - 8:hh - 7, :])

        # ---------- denominators + c_proj + residual -> xT ----------
        pst_cm = tc.tile_pool(name="pst", bufs=1, space="PSUM")
        pst = pst_cm.__enter__()
        s1 = pst.tile([1, 512], F32)
        s2 = pst.tile([1, 512], F32)
        with (
            tc.tile_pool(name="pcp", bufs=1) as pcp,
            tc.tile_pool(name="pcw", bufs=3) as cw,
            tc.tile_pool(name="pcr", bufs=2, space="PSUM") as csR,
            tc.tile_pool(name="pcps", bufs=1, space="PSUM") as cps,
        ):
            ysc = pcp.tile([128, NC, 512], BF16)
            for half in range(2):
                pjs = []
                for oi in range(4):
                    pjt = cps.tile([128, 512], F32, tag=f"cp{oi}")
                    pjs.append(pjt)
                for ct in range(NC):
                    if half == 0:
                        rb = csR.tile([128, 512], F32, tag="rb")
                        for hh in range(2):
                            h = ct * 2 + hh
                            for (c0, c1) in ((0, 256), (256, 512)):
                                nc.tensor.matmul(
                                    rb[hh * 64:hh * 64 + 64, c0:c1],
                                    w1_bf[0:1, h * 64:h * 64 + 64],
                                    s_bf[0:1, h * 512 + c0:h * 512 + c1],
                                    tile_position=(0, hh * 64),
                                    skip_group_check=True)
                        t1 = cw.tile([128, 512], F32, tag="yt1")
                        nc.vector.tensor_tensor(t1[:], yT_all[:, ct, :], rb[:],
                                                ALU.mult)
                        nc.vector.tensor_scalar(ysc[:, ct, :], t1[:], 1.0,
                                                ln1b(ct), ALU.mult, ALU.add)
                    for oi in range(4):
                        ot = half * 4 + oi
                        nc.tensor.matmul(
                            pjs[oi][:], cpw_sb[:, ct, ot * 128:(ot + 1) * 128],
                            ysc[:, ct, :], start=(ct == 0), stop=(ct == NC - 1))
                for oi in range(4):
                    ot = half * 4 + oi
                    t2 = cw.tile([128, 512], F32, tag="cpt")
                    nc.vector.tensor_scalar(t2[:], pjs[oi][:], 1.0, apb(ot),
                                            ALU.mult, ALU.add)
                    nc.vector.tensor_tensor(xT[:, ot, :], t2[:], qT_bf[:, ot, :],
                                            ALU.add)
                    sq = cw.tile([128, 512], BF16, tag="sq2")
                    nc.scalar.activation(sq[:], xT[:, ot, :], AF.Square)
                    nc.tensor.matmul(s1[:], ones_f[:, 0:1], xT[:, ot, :],
                                     start=(ot == 0), stop=(ot == NC - 1),
                                     skip_group_check=True)
                    nc.tensor.matmul(s2[:], ones_bf[:], sq[:],
                                     start=(ot == 0), stop=(ot == NC - 1),
                                     skip_group_check=True)

        py_cm.__exit__(None, None, None)

        # ---------- LN2 + MLP ----------
        with (
            tc.tile_pool(name="pm", bufs=1) as pm,
            tc.tile_pool(name="pmw", bufs=3) as mw,
            tc.tile_pool(name="pfw", bufs=8) as fwp,
            tc.tile_pool(name="pms", bufs=1, space="PSUM") as mps,
            tc.tile_pool(name="pma", bufs=2, space="PSUM") as mac,
        ):
            mu = pm.tile([1, 512], F32)
            nc.vector.tensor_scalar(mu[:], s1[:], 1.0 / C, None, ALU.mult)
            var = pm.tile([1, 512], F32)
            nc.vector.tensor_scalar(var[:], s2[:], 1.0 / C, EPS, ALU.mult, ALU.add)
            mu2 = pm.tile([1, 512], F32)
            nc.vector.tensor_tensor(mu2[:], mu[:], mu[:], ALU.mult)
            nc.vector.tensor_tensor(var[:], var[:], mu2[:], ALU.subtract)
            rstd2 = pm.tile([1, 512], F32)
            nc.scalar.activation(rstd2[:], var[:], AF.Ln)
            nc.scalar.activation(rstd2[:], rstd2[:], AF.Exp, scale=-0.5)
            gdum = pm.tile([1, 1], F32)
            nc.scalar.activation(gdum[:], rstd2[0:1, 0:1], AF.Gelu_apprx_tanh)
            nmr2 = pm.tile([1, 512], F32)
            nc.vector.tensor_tensor(nmr2[:], mu[:], rstd2[:], ALU.mult)
            nc.vector.tensor_scalar(nmr2[:], nmr2[:], -1.0, None, ALU.mult)
            rstd2b = pm.tile([1, 512], BF16)
            nc.vector.tensor_copy(rstd2b[:], rstd2[:])
            nmr2b = pm.tile([1, 512], BF16)
            nc.vector.tensor_copy(nmr2b[:], nmr2[:])

            zA = mps.tile([128, 512], F32, tag="zA")
            zB = mps.tile([128, 512], F32, tag="zB")
            nc.tensor.matmul(zA[:], ones_row[:], rstd2b[:], skip_group_check=True)
            nc.tensor.matmul(zB[:], ones_row[:], nmr2b[:], skip_group_check=True)

            z2 = pm.tile([128, NC, 512], BF16)
            for ct in range(NC):
                t1 = mw.tile([128, 512], F32, tag="z2t")
                nc.vector.tensor_tensor(t1[:], xT[:, ct, :], zA[:], ALU.mult)
                nc.vector.tensor_tensor(t1[:], t1[:], zB[:], ALU.add)
                nc.vector.tensor_scalar(z2[:, ct, :], t1[:],
                                        w2c(ct), b2c(ct), ALU.mult, ALU.add)

            mid = pm.tile([128, NF, 512], BF16)
            for ft in range(NF):
                fw = fwp.tile([128, NC, 128], BF16, tag="fw")
                nc.sync.dma_start(fw[:], fcw_d.ap()[ft])
                fp = mac.tile([128, 512], F32, tag="acc")
                for ct in range(NC):
                    nc.tensor.matmul(fp[:], fw[:, ct, :], z2[:, ct, :],
                                     start=(ct == 0), stop=(ct == NC - 1))
                nc.scalar.activation(mid[:, ft, :], fp[:], AF.Gelu_apprx_tanh,
                                     bias=fcb[:, ft:ft + 1])

            outT = pm.tile([128, NC, 512], F32)
            ons = [pm.tile([128, C], F32, name=f"on{i}", tag=f"on{i}") for i in range(NSLOT)]
            for ot in range(NC):
                pw = mw.tile([128, NF, 128], BF16, tag="pw")
                nc.sync.dma_start(pw[:], pjw_d.ap()[ot])
                pacc = mac.tile([128, 512], F32, tag="acc")
                for ft in range(NF):
                    nc.tensor.matmul(pacc[:], pw[:, ft, :], mid[:, ft, :],
                                     start=(ft == 0), stop=(ft == NF - 1))
                t3 = mw.tile([128, 512], F32, tag="ot3")
                nc.vector.tensor_scalar(t3[:], pacc[:], 1.0, pjb(ot),
                                        ALU.mult, ALU.add)
                nc.vector.tensor_tensor(outT[:, ot, :], t3[:], xT[:, ot, :], ALU.add)
                # out transposes ride behind proj, per-ot
                pot = mac.tile([128, 4, 128], F32, tag="po")
                for i in range(NSLOT):
                    nc.tensor.transpose(pot[:, i, :],
                                        outT[:, ot, i * 128:(i + 1) * 128],
                                        ident[:])
                for i in range(NSLOT):
                    if i % 2 == 0:
                        nc.scalar.copy(ons[i][:, ot * 128:(ot + 1) * 128],
                                       pot[:, i, :])
                    else:
                        nc.vector.tensor_copy(ons[i][:, ot * 128:(ot + 1) * 128],
                                              pot[:, i, :])
                if ot == 3:
                    for i in range(NSLOT):
                        nc.sync.dma_start(out_d.ap()[i][:, 0:512],
                                          ons[i][:, 0:512])
            for i in range(NSLOT):
                nc.sync.dma_start(out_d.ap()[i][:, 512:1024],
                                  ons[i][:, 512:1024])

        pst_cm.__exit__(None, None, None)

    nc.compile()
    return nc


def _host_prep(inputs):
    q = np.asarray(inputs["q"], np.float32)
    k = np.asarray(inputs["k"], np.float32)
    v = np.asarray(inputs["v"], np.float32)

    bf16 = ml_dtypes.bfloat16
    cpwt = np.ascontiguousarray(
        np.asarray(inputs["attn_proj_w"], np.float32).T.reshape(NC, 128, C)
        .transpose(1, 0, 2)).astype(bf16)             # [p, ct, o]
    fcwt = np.ascontiguousarray(
        np.asarray(inputs["fc_w"], np.float32).T.reshape(NC, 128, NF, 128)
        .transpose(2, 1, 0, 3)).astype(bf16)          # [ft, p, ct, f]
    pjwt = np.ascontiguousarray(
        np.asarray(inputs["proj_w"], np.float32).T.reshape(NF, 128, NC, 128)
        .transpose(2, 1, 0, 3)).astype(bf16)          # [ot, p, ft, o]

    vecs = np.ascontiguousarray(np.stack(
        [np.asarray(inputs["ln1_w"], np.float32),
         np.asarray(inputs["ln1_b"], np.float32),
         np.asarray(inputs["attn_proj_b"], np.float32),
         np.asarray(inputs["proj_b"], np.float32),
         np.asarray(inputs["ln2_w"], np.float32),
         np.asarray(inputs["ln2_b"], np.float32)], axis=1))
    w1row = np.asarray(inputs["ln1_w"], np.float32)[None, :].astype(ml_dtypes.bfloat16)
    fcb = np.ascontiguousarray(np.asarray(inputs["fc_b"], np.float32))

    in_maps, slot_map = [], []
    for c in range(N_CORES):
        b, r = c // 4, c % 4
        slots = [r, 7 - r, 8 + r, 15 - r]
        slot_map.append((b, slots))
        qs = q[b].reshape(NT, 128, C)[slots]
        # multiplicative causal masks per band chunk: keep iff
        # key (p + 512*i + 128*t) <= query (128*a_i + j).
        # mask[:, part, :, 0:128] = first slot's band,
        # mask[:, part, :, 128:256] = second slot's band.
        mask = np.ones((128, 2, 4, 256), np.float32)
        p = np.arange(128)[:, None, None]
        t = np.arange(4)[None, :, None]
        j = np.arange(128)[None, None, :]
        for part, (ba, cl) in enumerate(((0, 1), (2, 3))):
            mask[:, part, :, 0:128] = (
                (p + 512 * ba + 128 * t) <= (128 * slots[ba] + j))
            mask[:, part, :, 128:256] = (
                (p + 512 * cl + 128 * t) <= (128 * slots[cl] + j))
        in_maps.append({
            "q_s": np.ascontiguousarray(qs).astype(ml_dtypes.bfloat16),
            "k_f": np.ascontiguousarray(k[b].reshape(NT, 128, C)).astype(ml_dtypes.bfloat16),
            "v_f": np.ascontiguousarray(v[b].reshape(NT, 128, C)).astype(ml_dtypes.bfloat16),
            "mask": mask.astype(ml_dtypes.bfloat16),
            "vecs": vecs, "w1row": w1row,
            "cpwt": cpwt, "fcwt": fcwt, "pjwt": pjwt, "fcb": fcb,
        })
    return in_maps, slot_map


def kernel(**inputs):
    if "nc" not in _CACHE:
        _CACHE["nc"] = build()
    nc = _CACHE["nc"]
    in_maps, slot_map = _host_prep(inputs)
    res = run_bass_kernel_spmd(nc, in_maps, core_ids=list(range(N_CORES)))
    out = np.empty((B, T, C), np.float32)
    for c in range(N_CORES):
        b, slots = slot_map[c]
        o = res.results[c]["out"]
        for i, a in enumerate(slots):
            out[b, a * 128:(a + 1) * 128, :] = o[i]
    return out


# revision 45
# speedup vs baseline: 1.0998x; 1.0212x over previous
"""Trainium2 Bass kernel for a dense pre-LN transformer block (B=2, T=2048, C=1024, H=16).

Sharding: zero-collective sequence parallelism over 8 cores. Core c handles
batch b=c//4 and query tiles (slots) {r, 7-r, 8+r, 15-r} (r=c%4, 128 rows
each). The slot-position windows align with 4-tile causal bands: slot
position i always has its diagonal inside key tiles [4i, 4i+4), so the
program is SPMD-identical while masks are data (paged-mask offsets).

Per head, attention runs in two parts: part A covers query cols 0:256
(slots 0,1; key tiles 0..7), part B covers cols 256:512 (slots 2,3; key
tiles 0..15), so part A can start once k/v tiles 0..7 are layer-normed.
Heads are processed in PAIRS with QK matmuls interleaved between the two
heads: the pair occupies disjoint 64-row groups of the PE array, so both
the LDWEIGHTS and the MATMULs of the two heads run concurrently (~2x QK).
Causality: exact spans per chunk; only the 4-tile diagonal bands get a
bf16 mask multiply. Softmax denominators ride along as an extra
ones-column in the v stationary; their reciprocals (fast DVE approx) and
the s_bf gather are hoisted into the part-B pipeline so c_proj can start
immediately after attention. LN2 token-stat matmuls accumulate per-ot
inside c_proj. Activation tables are pinned so Exp/Ln share one set
(zero mid-kernel table swaps except one load for gelu).

Numerics: all matmuls bf16 with fp32 PSUM accumulation; residuals/LN math
fp32 (fp8 was evaluated and rejected: e4m3 matmul quantization costs
1.3-2e-2 of max-rel-err against a 2e-2 budget).
"""

import sys
import functools

sys.path.insert(0, "/opt/trn_rl_repo")

import numpy as np
import ml_dtypes

import concourse.bass as bass
import concourse.bacc as bacc
import concourse.mybir as mybir
import concourse.tile as tile
from concourse.bass_utils import run_bass_kernel_spmd

# Pin Exp/Ln to the one table set that holds both, so the act-table
# placement pass cannot thrash between exp_and_others and natural_log
# (each swap costs ~1.3us and stalls the attention softmax pipeline).
# Indices into act_info.json are preserved; only set membership as seen
# by the placement pass is narrowed.
_AF = mybir.ActivationFunctionType


@functools.cache
def _pinned_act_tables(arch):
    import concourse.hw_specs as hw_specs
    out = {}
    for name, fns in hw_specs.get_activation_tables(arch).items():
        fns = set(fns)
        if name != "natural_log_exp_and_others":
            fns.discard(_AF.Exp)
            fns.discard(_AF.Ln)
        out[name] = fns
    return out


bacc.get_activation_tables = _pinned_act_tables

F32 = mybir.dt.float32
BF16 = mybir.dt.bfloat16
AF = mybir.ActivationFunctionType
ALU = mybir.AluOpType

B, T, C, H, D = 2, 2048, 1024, 16, 64
NT = T // 128          # 16 key tiles
NC = C // 128          # 8 channel tiles
NF = 4 * C // 128      # 32 fc tiles
NSLOT = 4
N_CORES = 8
EPS = 1e-5
SCALE = 1.0 / 8.0      # 1/sqrt(D)

_CACHE = {}


def build():
    nc = bacc.Bacc("TRN2", target_bir_lowering=False, debug=False,
                   num_devices=N_CORES)

    q_d = nc.dram_tensor("q_s", [NSLOT, 128, C], BF16, kind="ExternalInput")
    k_d = nc.dram_tensor("k_f", [NT, 128, C], BF16, kind="ExternalInput")
    v_d = nc.dram_tensor("v_f", [NT, 128, C], BF16, kind="ExternalInput")
    mask_d = nc.dram_tensor("mask", [128, 2, 4, 256], BF16, kind="ExternalInput")
    vecs_d = nc.dram_tensor("vecs", [C, 6], F32, kind="ExternalInput")
    w1_d = nc.dram_tensor("w1row", [1, C], BF16, kind="ExternalInput")
    cpw_d = nc.dram_tensor("cpwt", [128, NC, C], BF16, kind="ExternalInput")
    fcw_d = nc.dram_tensor("fcwt", [NF, 128, NC, 128], BF16, kind="ExternalInput")
    pjw_d = nc.dram_tensor("pjwt", [NC, 128, NF, 128], BF16, kind="ExternalInput")
    fcb_d = nc.dram_tensor("fcb", [4 * C], F32, kind="ExternalInput")
    out_d = nc.dram_tensor("out", [NSLOT, 128, C], F32, kind="ExternalOutput")

    with tile.TileContext(nc) as tc:
      with tc.tile_pool(name="pg", bufs=1) as pg:
        # ---- persistent constants (gpsimd DMA queue: keeps the sync
        # queue free for the startup q/k/v tile loads) ----
        vecs = pg.tile([128, NC, 6], F32)   # ln1w ln1b apb pjb w2 b2
        nc.gpsimd.dma_start(vecs[:], vecs_d.ap().rearrange("(ct p) v -> p ct v", p=128))
        masks = pg.tile([128, 2, 4, 256], BF16)
        nc.gpsimd.dma_start(masks[:], mask_d.ap())
        w1_bf = pg.tile([1, C], BF16)
        nc.gpsimd.dma_start(w1_bf[:], w1_d.ap())
        fcb = pg.tile([128, NF], F32)
        nc.gpsimd.dma_start(fcb[:], fcb_d.ap().rearrange("(ft p) -> p ft", p=128))

        ones_f = pg.tile([128, 128], F32)
        nc.gpsimd.memset(ones_f[:], 1.0)
        ident = pg.tile([128, 128], F32)
        nc.gpsimd.affine_select(ident[:], ones_f[:], [[1, 128]], ALU.is_equal,
                                0.0, channel_multiplier=-1)
        ones128_bf = pg.tile([128, 128], BF16)
        nc.gpsimd.memset(ones128_bf[:], 1.0)
        ident_bf = pg.tile([128, 128], BF16)
        nc.gpsimd.affine_select(ident_bf[:], ones128_bf[:], [[1, 128]], ALU.is_equal,
                                0.0, channel_multiplier=-1)
        ones_bf = pg.tile([128, 1], BF16)
        nc.gpsimd.memset(ones_bf[:], 1.0)
        ones_row = pg.tile([1, 128], BF16)
        nc.gpsimd.memset(ones_row[:], 1.0)

        ln1w = lambda ct: vecs[:, ct, 0:1]
        ln1b = lambda ct: vecs[:, ct, 1:2]
        apb = lambda ct: vecs[:, ct, 2:3]
        pjb = lambda ct: vecs[:, ct, 3:4]
        w2c = lambda ct: vecs[:, ct, 4:5]
        b2c = lambda ct: vecs[:, ct, 5:6]

        # ---- cross-phase tensors ----
        qT_bf = pg.tile([128, NC, 512], BF16)   # LN1(q)^T w/ w,b (QK rhs + residual)
        xT = pg.tile([128, NC, 512], F32)       # attn residual output (C-major)

        py_cm = tc.tile_pool(name="py", bufs=1)
        py = py_cm.__enter__()
        yT_all = py.tile([128, NC, 512], F32)   # raw attention out (pre 1/s, w1, b1)
        s_all = py.tile([H, 512], F32)          # softmax denominators
        s_bf = py.tile([1, H * 512], BF16)      # reciprocals, gathered on one row
        srec_b = py.tile([H, 512], BF16)
        s_allB2 = py.tile([8, 256], F32)        # part-B denoms, heads 8..15
        srec_b2 = py.tile([8, 256], BF16)
        sA_rec = py.tile([H, 256], F32)         # fast-recip scratch (part A)
        cpw_sb = py.tile([128, NC, C], BF16)    # c_proj weights, preloaded
        nc.gpsimd.dma_start(cpw_sb[:], cpw_d.ap())

        with tc.tile_pool(name="pa", bufs=1) as pa:
            kT = pa.tile([128, NC, T], BF16)          # LN1(k)^T w/ w,b
            # v_ext cols: [v0..v63 | ones] — AV output rows 0..63 = y,
            # row 64 = softmax denominator.
            v_ext = pa.tile([128, NT, H, 65], BF16)
            for tt in range(NT):
                nc.gpsimd.memset(v_ext[:, tt, :, 64:65], 1.0)

            with (
                tc.tile_pool(name="pln", bufs=8) as pl,
                tc.tile_pool(name="plz", bufs=2) as plz,
                tc.tile_pool(name="pla", bufs=3) as pla,
                tc.tile_pool(name="pat", bufs=2) as pat,
                tc.tile_pool(name="psA", bufs=2, space="PSUM") as psA,
                tc.tile_pool(name="psB", bufs=2, space="PSUM") as psB,
                tc.tile_pool(name="psY", bufs=2, space="PSUM") as psY,
            ):
                # ---------- LN1 group: load, stats, rstd, normalize ----------
                def ln_group(src_d, tts, kind):
                    n = len(tts)
                    xs = []
                    agg = pla.tile([128, 8, 2], F32, tag="agg")
                    for gi, tt in enumerate(tts):
                        x = pl.tile([128, C], BF16, tag="xin")
                        nc.sync.dma_start(x[:], src_d.ap()[tt])
                        st6 = pl.tile([128, 2, 6], F32, tag="st6")
                        nc.vector.bn_stats(st6[:, 0, :], x[:, 0:512])
                        nc.vector.bn_stats(st6[:, 1, :], x[:, 512:1024])
                        nc.vector.bn_aggr(agg[:, gi, :], st6[:])
                        xs.append(x)
                    veps = pla.tile([128, 8], F32, tag="veps")
                    nc.vector.tensor_scalar(veps[:, 0:n], agg[:, 0:n, 1], EPS, None,
                                            ALU.add)
                    rstd = pla.tile([128, 8], F32, tag="rstd")
                    nc.scalar.activation(rstd[:, 0:n], veps[:, 0:n], AF.Ln)
                    nc.scalar.activation(rstd[:, 0:n], rstd[:, 0:n], AF.Exp,
                                         scale=-0.5)
                    nmr = pla.tile([128, 8], F32, tag="nmr")
                    nc.vector.tensor_tensor(nmr[:, 0:n], agg[:, 0:n, 0], rstd[:, 0:n],
                                            ALU.mult)
                    nc.vector.tensor_scalar(nmr[:, 0:n], nmr[:, 0:n], -1.0, None,
                                            ALU.mult)
                    if kind == "v":
                        for gi, tt in enumerate(tts):
                            nc.vector.tensor_scalar(
                                v_ext[:, tt, :, 0:64],
                                xs[gi][:].rearrange("p (h d) -> p h d", h=H),
                                rstd[:, gi:gi + 1], nmr[:, gi:gi + 1],
                                ALU.mult, ALU.add)
                        return
                    # q/k: normalize -> transpose -> evacuate with w,b
                    dstT, col0 = (qT_bf, 0) if kind == "q" else (kT, tts[0] * 128)
                    late = kind == "k" and tts[0] >= 8
                    zs = []
                    for gi, tt in enumerate(tts):
                        z = plz.tile([128, C], BF16, tag=f"z{gi % 4}")
                        nc.vector.tensor_scalar(z[:], xs[gi][:],
                                                rstd[:, gi:gi + 1], nmr[:, gi:gi + 1],
                                                ALU.mult, ALU.add)
                        zs.append(z)
                    for half in range(n // 4):
                        for ct in range(NC):
                            ps = psB.tile([128, 4, 128], F32, tag="p128")
                            pv = ps[:].bitcast(BF16)[:, :, 0:128]
                            for gi in range(4):
                                nc.tensor.transpose(
                                    pv[:, gi, :],
                                    zs[half * 4 + gi][:, ct * 128:(ct + 1) * 128],
                                    ident_bf[:])
                            dst = dstT[:, ct, col0 + half * 512:col0 + half * 512 + 512]
                            if late and ct % 2 == 0:
                                # split evacs ACT/DVE while softmax runs
                                nc.vector.tensor_scalar(dst, pv[:], ln1w(ct),
                                                        ln1b(ct), ALU.mult, ALU.add)
                            else:
                                nc.scalar.activation(dst, pv[:], AF.Identity,
                                                     bias=ln1b(ct), scale=ln1w(ct))

                # ---------- attention, head pair (h0, h0+1) ----------
                # QK matmuls interleave the two heads (disjoint 64-row PE
                # groups -> concurrent LDWEIGHTS+MATMUL for the pair).
                def qk_a_pair(h0):
                    hs = (h0, h0 + 1)
                    sc0 = {}
                    for h in hs:
                        sc0[h] = psA.tile([128, 4, 256], F32, name="sc0", tag="sc256")
                    for t in range(4):
                        for h in hs:
                            ct, sel = h // 2, (h % 2) * 64
                            nc.tensor.matmul(sc0[h][:, t, :],
                                             kT[sel:sel + 64, ct, t * 128:(t + 1) * 128],
                                             qT_bf[sel:sel + 64, ct, 0:256],
                                             tile_position=(sel, 0),
                                             skip_group_check=True)
                    att0 = {}
                    for h in hs:
                        a = pat.tile([128, 4, 256], BF16, tag="attA0")
                        nc.scalar.activation(a[:], sc0[h][:], AF.Exp, scale=SCALE)
                        nc.vector.tensor_tensor(a[:, :, 0:128], a[:, :, 0:128],
                                                masks[:, 0, :, 0:128], ALU.mult)
                        att0[h] = a
                    sc1 = {}
                    for h in hs:
                        sc1[h] = psB.tile([128, 4, 128], F32, name="sc1", tag="p128")
                    for t in range(4, 8):
                        for h in hs:
                            ct, sel = h // 2, (h % 2) * 64
                            nc.tensor.matmul(sc1[h][:, t - 4, :],
                                             kT[sel:sel + 64, ct, t * 128:(t + 1) * 128],
                                             qT_bf[sel:sel + 64, ct, 128:256],
                                             tile_position=(sel, 0),
                                             skip_group_check=True)
                    att1 = {}
                    for h in hs:
                        a = pat.tile([128, 4, 128], BF16, tag="attA1")
                        nc.scalar.activation(a[:], sc1[h][:], AF.Exp, scale=SCALE)
                        nc.vector.tensor_tensor(a[:], a[:],
                                                masks[:, 0, :, 128:256], ALU.mult)
                        att1[h] = a
                    return {h: (att0[h], att1[h]) for h in hs}

                def av_a(h, att0, att1):
                    yp = psY.tile([65, 512], F32, tag="yp")
                    vx = lambda t: v_ext[:, t, h, :]
                    for t in range(3):
                        nc.tensor.matmul(yp[:, 0:256], vx(t), att0[:, t, :],
                                         start=(t == 0), stop=False,
                                         skip_group_check=True)
                    nc.tensor.matmul(yp[:, 0:128], vx(3), att0[:, 3, 0:128],
                                     start=False, stop=True, skip_group_check=True)
                    nc.tensor.matmul(yp[:, 128:256], vx(3), att0[:, 3, 128:256],
                                     start=False, stop=False, skip_group_check=True)
                    for t in range(4, 8):
                        nc.tensor.matmul(yp[:, 128:256], vx(t), att1[:, t - 4, :],
                                         start=False, stop=(t == 7),
                                         skip_group_check=True)
                    return yp

                def qk_b_pair(h0):
                    hs = (h0, h0 + 1)
                    att0 = {h: pat.tile([128, 8, 256], BF16, name="attB0", tag="attB0") for h in hs}
                    for half in range(2):
                        sc = {}
                        for h in hs:
                            sc[h] = psA.tile([128, 4, 256], F32, name="sc", tag="sc256")
                        for tl in range(4):
                            t = half * 4 + tl
                            for h in hs:
                                ct, sel = h // 2, (h % 2) * 64
                                nc.tensor.matmul(sc[h][:, tl, :],
                                                 kT[sel:sel + 64, ct, t * 128:(t + 1) * 128],
                                                 qT_bf[sel:sel + 64, ct, 256:512],
                                                 tile_position=(sel, 0),
                                                 skip_group_check=True)
                        for h in hs:
                            nc.scalar.activation(att0[h][:, half * 4:half * 4 + 4, :],
                                                 sc[h][:], AF.Exp, scale=SCALE)
                    sc1 = {}
                    for h in hs:
                        sc1[h] = psA.tile([128, 4, 256], F32, name="sc1b", tag="sc256")
                    for t in range(8, 12):
                        for h in hs:
                            ct, sel = h // 2, (h % 2) * 64
                            nc.tensor.matmul(sc1[h][:, t - 8, :],
                                             kT[sel:sel + 64, ct, t * 128:(t + 1) * 128],
                                             qT_bf[sel:sel + 64, ct, 256:512],
                                             tile_position=(sel, 0),
                                             skip_group_check=True)
                    att1 = {}
                    for h in hs:
                        a = pat.tile([128, 4, 256], BF16, tag="attB1")
                        nc.scalar.activation(a[:], sc1[h][:], AF.Exp, scale=SCALE)
                        nc.vector.tensor_tensor(a[:, :, 0:128], a[:, :, 0:128],
                                                masks[:, 1, :, 0:128], ALU.mult)
                        att1[h] = a
                    sc2 = {}
                    for h in hs:
                        sc2[h] = psB.tile([128, 4, 128], F32, name="sc2", tag="p128")
                    for t in range(12, 16):
                        for h in hs:
                            ct, sel = h // 2, (h % 2) * 64
                            nc.tensor.matmul(sc2[h][:, t - 12, :],
                                             kT[sel:sel + 64, ct, t * 128:(t + 1) * 128],
                                             qT_bf[sel:sel + 64, ct, 384:512],
                                             tile_position=(sel, 0),
                                             skip_group_check=True)
                    att2 = {}
                    for h in hs:
                        a = pat.tile([128, 4, 128], BF16, tag="attB2")
                        nc.scalar.activation(a[:], sc2[h][:], AF.Exp, scale=SCALE)
                        nc.vector.tensor_tensor(a[:], a[:],
                                                masks[:, 1, :, 128:256], ALU.mult)
                        att2[h] = a
                    return {h: (att0[h], att1[h], att2[h]) for h in hs}

                def av_b(h, att0, att1, att2):
                    yp = psY.tile([65, 512], F32, tag="yp")
                    vx = lambda t: v_ext[:, t, h, :]
                    for t in range(8):
                        nc.tensor.matmul(yp[:, 0:256], vx(t), att0[:, t, :],
                                         start=(t == 0), stop=False,
                                         skip_group_check=True)
                    for t in range(8, 11):
                        nc.tensor.matmul(yp[:, 0:256], vx(t), att1[:, t - 8, :],
                                         start=False, stop=False,
                                         skip_group_check=True)
                    nc.tensor.matmul(yp[:, 0:128], vx(11), att1[:, 3, 0:128],
                                     start=False, stop=True, skip_group_check=True)
                    nc.tensor.matmul(yp[:, 128:256], vx(11), att1[:, 3, 128:256],
                                     start=False, stop=False, skip_group_check=True)
                    for t in range(12, 16):
                        nc.tensor.matmul(yp[:, 128:256], vx(t), att2[:, t - 12, :],
                                         start=False, stop=(t == 15),
                                         skip_group_check=True)
                    return yp

                def evac(h, yp, c0):
                    ct, sel = h // 2, (h % 2) * 64
                    if c0 == 0:
                        nc.scalar.copy(yT_all[sel:sel + 64, ct, c0:c0 + 256],
                                       yp[0:64, 0:256])
                    else:
                        nc.vector.tensor_copy(yT_all[sel:sel + 64, ct, c0:c0 + 256],
                                              yp[0:64, 0:256])
                    srow = pla.tile([65, 256], F32, tag="srow")
                    nc.vector.tensor_copy(srow[64:65, :], yp[64:65, 0:256])
                    if c0 == 0:
                        nc.gpsimd.dma_start(s_all[h:h + 1, 0:256],
                                            srow[64:65, :])
                    elif h < 8:
                        nc.gpsimd.dma_start(s_all[h:h + 1, 256:512],
                                            srow[64:65, :])
                    else:
                        nc.gpsimd.dma_start(s_allB2[h - 8:h - 7, :],
                                            srow[64:65, :])

                # ---------- emission: LN groups + paired-head pipeline ----------
                ln_group(q_d, range(0, 4), "q")
                ln_group(k_d, range(0, 4), "k")
                ln_group(k_d, range(4, 8), "k")
                ln_group(v_d, range(0, 4), "v")
                ln_group(v_d, range(4, 8), "v")

                apair = {}
                for j in range(9):
                    if j < 8:
                        apair[j] = qk_a_pair(2 * j)
                    if j >= 1:
                        prev = apair.pop(j - 1)
                        for hh in (2 * (j - 1), 2 * (j - 1) + 1):
                            yp = av_a(hh, *prev[hh])
                            evac(hh, yp, 0)
                    if j == 1:
                        ln_group(k_d, range(8, 16), "k")
                    if j == 4:
                        ln_group(v_d, range(8, 16), "v")

                bpair = {}
                for j in range(9):
                    if j < 8:
                        bpair[j] = qk_b_pair(2 * j)
                    if j == 1:
                        # part-A denominators: fast recip + gather while B runs
                        nc.vector.reciprocal_approx_fast(sA_rec[:],
                                                         s_all[:, 0:256])
                        nc.vector.tensor_copy(srec_b[:, 0:256], sA_rec[:])
                        for hh in range(H):
                            nc.gpsimd.dma_start(s_bf[0:1, hh * 512:hh * 512 + 256],
                                                srec_b[hh:hh + 1, 0:256])
                    if j >= 1:
                        prev = bpair.pop(j - 1)
                        for hh in (2 * (j - 1), 2 * (j - 1) + 1):
                            yp = av_b(hh, *prev[hh])
                            evac(hh, yp, 256)
                        if 2 * (j - 1) + 1 == 7:
                            nc.vector.reciprocal_approx_fast(sA_rec[0:8, :],
                                                             s_all[0:8, 256:512])
                            nc.vector.tensor_copy(srec_b[0:8, 256:512],
                                                  sA_rec[0:8, :])
                            for hh in range(8):
                                nc.gpsimd.dma_start(
                                    s_bf[0:1, hh * 512 + 256:(hh + 1) * 512],
                                    srec_b[hh:hh + 1, 256:512])
                        elif 2 * (j - 1) + 1 == 15:
                            nc.vector.reciprocal_approx_fast(sA_rec[0:8, :],
                                                             s_allB2[:])
                            nc.vector.tensor_copy(srec_b2[:], sA_rec[0:8, :])
                            for hh in range(8, 16):
                                nc.gpsimd.dma_start(
                                    s_bf[0:1, hh * 512 + 256:(hh + 1) * 512],
                                    srec_b2[hh - 8:hh - 7, :])

        # ---------- denominators + c_proj + residual -> xT ----------
        pst_cm = tc.tile_pool(name="pst", bufs=1, space="PSUM")
        pst = pst_cm.__enter__()
        s1 = pst.tile([1, 512], F32)
        s2 = pst.tile([1, 512], F32)
        with (
            tc.tile_pool(name="pcp", bufs=1) as pcp,
            tc.tile_pool(name="pcw", bufs=3) as cw,
            tc.tile_pool(name="pcr", bufs=2, space="PSUM") as csR,
            tc.tile_pool(name="pcps", bufs=1, space="PSUM") as cps,
        ):
            ysc = pcp.tile([128, NC, 512], BF16)
            for half in range(2):
                pjs = []
                for oi in range(4):
                    pjt = cps.tile([128, 512], F32, tag=f"cp{oi}")
                    pjs.append(pjt)
                for ct in range(NC):
                    if half == 0:
                        rb = csR.tile([128, 512], F32, tag="rb")
                        for hh in range(2):
                            h = ct * 2 + hh
                            for (c0, c1) in ((0, 256), (256, 512)):
                                nc.tensor.matmul(
                                    rb[hh * 64:hh * 64 + 64, c0:c1],
                                    w1_bf[0:1, h * 64:h * 64 + 64],
                                    s_bf[0:1, h * 512 + c0:h * 512 + c1],
                                    tile_position=(0, hh * 64),
                                    skip_group_check=True)
                        t1 = cw.tile([128, 512], F32, tag="yt1")
                        nc.vector.tensor_tensor(t1[:], yT_all[:, ct, :], rb[:],
                                                ALU.mult)
                        nc.vector.tensor_scalar(ysc[:, ct, :], t1[:], 1.0,
                                                ln1b(ct), ALU.mult, ALU.add)
                    for oi in range(4):
                        ot = half * 4 + oi
                        nc.tensor.matmul(
                            pjs[oi][:], cpw_sb[:, ct, ot * 128:(ot + 1) * 128],
                            ysc[:, ct, :], start=(ct == 0), stop=(ct == NC - 1))
                for oi in range(4):
                    ot = half * 4 + oi
                    t2 = cw.tile([128, 512], F32, tag="cpt")
                    nc.vector.tensor_scalar(t2[:], pjs[oi][:], 1.0, apb(ot),
                                            ALU.mult, ALU.add)
                    nc.vector.tensor_tensor(xT[:, ot, :], t2[:], qT_bf[:, ot, :],
                                            ALU.add)
                    sq = cw.tile([128, 512], BF16, tag="sq2")
                    nc.scalar.activation(sq[:], xT[:, ot, :], AF.Square)
                    nc.tensor.matmul(s1[:], ones_f[:, 0:1], xT[:, ot, :],
                                     start=(ot == 0), stop=(ot == NC - 1),
                                     skip_group_check=True)
                    nc.tensor.matmul(s2[:], ones_bf[:], sq[:],
                                     start=(ot == 0), stop=(ot == NC - 1),
                                     skip_group_check=True)

        py_cm.__exit__(None, None, None)

        # ---------- LN2 + MLP ----------
        with (
            tc.tile_pool(name="pm", bufs=1) as pm,
            tc.tile_pool(name="pmw", bufs=3) as mw,
            tc.tile_pool(name="pfw", bufs=8) as fwp,
            tc.tile_pool(name="pms", bufs=1, space="PSUM") as mps,
            tc.tile_pool(name="pma", bufs=2, space="PSUM") as mac,
        ):
            mu = pm.tile([1, 512], F32)
            nc.vector.tensor_scalar(mu[:], s1[:], 1.0 / C, None, ALU.mult)
            var = pm.tile([1, 512], F32)
            nc.vector.tensor_scalar(var[:], s2[:], 1.0 / C, EPS, ALU.mult, ALU.add)
            mu2 = pm.tile([1, 512], F32)
            nc.vector.tensor_tensor(mu2[:], mu[:], mu[:], ALU.mult)
            nc.vector.tensor_tensor(var[:], var[:], mu2[:], ALU.subtract)
            rstd2 = pm.tile([1, 512], F32)
            nc.scalar.activation(rstd2[:], var[:], AF.Ln)
            nc.scalar.activation(rstd2[:], rstd2[:], AF.Exp, scale=-0.5)
            gdum = pm.tile([1, 1], F32)
            nc.scalar.activation(gdum[:], rstd2[0:1, 0:1], AF.Gelu_apprx_tanh)
            nmr2 = pm.tile([1, 512], F32)
            nc.vector.tensor_tensor(nmr2[:], mu[:], rstd2[:], ALU.mult)
            nc.vector.tensor_scalar(nmr2[:], nmr2[:], -1.0, None, ALU.mult)
            rstd2b = pm.tile([1, 512], BF16)
            nc.vector.tensor_copy(rstd2b[:], rstd2[:])
            nmr2b = pm.tile([1, 512], BF16)
            nc.vector.tensor_copy(nmr2b[:], nmr2[:])

            zA = mps.tile([128, 512], F32, tag="zA")
            zB = mps.tile([128, 512], F32, tag="zB")
            nc.tensor.matmul(zA[:], ones_row[:], rstd2b[:], skip_group_check=True)
            nc.tensor.matmul(zB[:], ones_row[:], nmr2b[:], skip_group_check=True)

            z2 = pm.tile([128, NC, 512], BF16)
            for ct in range(NC):
                t1 = mw.tile([128, 512], F32, tag="z2t")
                nc.vector.tensor_tensor(t1[:], xT[:, ct, :], zA[:], ALU.mult)
                nc.vector.tensor_tensor(t1[:], t1[:], zB[:], ALU.add)
                nc.vector.tensor_scalar(z2[:, ct, :], t1[:],
                                        w2c(ct), b2c(ct), ALU.mult, ALU.add)

            mid = pm.tile([128, NF, 512], BF16)
            for ft in range(NF):
                fw = fwp.tile([128, NC, 128], BF16, tag="fw")
                nc.sync.dma_start(fw[:], fcw_d.ap()[ft])
                fp = mac.tile([128, 512], F32, tag="acc")
                for ct in range(NC):
                    nc.tensor.matmul(fp[:], fw[:, ct, :], z2[:, ct, :],
                                     start=(ct == 0), stop=(ct == NC - 1))
                nc.scalar.activation(mid[:, ft, :], fp[:], AF.Gelu_apprx_tanh,
                                     bias=fcb[:, ft:ft + 1])

            outT = pm.tile([128, NC, 512], F32)
            ons = [pm.tile([128, C], F32, name=f"on{i}", tag=f"on{i}") for i in range(NSLOT)]
            for ot in range(NC):
                pw = mw.tile([128, NF, 128], BF16, tag="pw")
                nc.sync.dma_start(pw[:], pjw_d.ap()[ot])
                pacc = mac.tile([128, 512], F32, tag="acc")
                for ft in range(NF):
                    nc.tensor.matmul(pacc[:], pw[:, ft, :], mid[:, ft, :],
                                     start=(ft == 0), stop=(ft == NF - 1))
                t3 = mw.tile([128, 512], F32, tag="ot3")
                nc.vector.tensor_scalar(t3[:], pacc[:], 1.0, pjb(ot),
                                        ALU.mult, ALU.add)
                nc.vector.tensor_tensor(outT[:, ot, :], t3[:], xT[:, ot, :], ALU.add)
                # out transposes ride behind proj, per-ot
                pot = mac.tile([128, 4, 128], F32, tag="po")
                for i in range(NSLOT):
                    nc.tensor.transpose(pot[:, i, :],
                                        outT[:, ot, i * 128:(i + 1) * 128],
                                        ident[:])
                for i in range(NSLOT):
                    if i % 2 == 0:
                        nc.scalar.copy(ons[i][:, ot * 128:(ot + 1) * 128],
                                       pot[:, i, :])
                    else:
                        nc.vector.tensor_copy(ons[i][:, ot * 128:(ot + 1) * 128],
                                              pot[:, i, :])
                if ot == 3:
                    for i in range(NSLOT):
                        nc.sync.dma_start(out_d.ap()[i][:, 0:512],
                                          ons[i][:, 0:512])
            for i in range(NSLOT):
                nc.sync.dma_start(out_d.ap()[i][:, 512:1024],
                                  ons[i][:, 512:1024])

        pst_cm.__exit__(None, None, None)

    nc.compile()
    return nc


def _host_prep(inputs):
    q = np.asarray(inputs["q"], np.float32)
    k = np.asarray(inputs["k"], np.float32)
    v = np.asarray(inputs["v"], np.float32)

    bf16 = ml_dtypes.bfloat16
    cpwt = np.ascontiguousarray(
        np.asarray(inputs["attn_proj_w"], np.float32).T.reshape(NC, 128, C)
        .transpose(1, 0, 2)).astype(bf16)             # [p, ct, o]
    fcwt = np.ascontiguousarray(
        np.asarray(inputs["fc_w"], np.float32).T.reshape(NC, 128, NF, 128)
        .transpose(2, 1, 0, 3)).astype(bf16)          # [ft, p, ct, f]
    pjwt = np.ascontiguousarray(
        np.asarray(inputs["proj_w"], np.float32).T.reshape(NF, 128, NC, 128)
        .transpose(2, 1, 0, 3)).astype(bf16)          # [ot, p, ft, o]

    vecs = np.ascontiguousarray(np.stack(
        [np.asarray(inputs["ln1_w"], np.float32),
         np.asarray(inputs["ln1_b"], np.float32),
         np.asarray(inputs["attn_proj_b"], np.float32),
         np.asarray(inputs["proj_b"], np.float32),
         np.asarray(inputs["ln2_w"], np.float32),
         np.asarray(inputs["ln2_b"], np.float32)], axis=1))
    w1row = np.asarray(inputs["ln1_w"], np.float32)[None, :].astype(ml_dtypes.bfloat16)
    fcb = np.ascontiguousarray(np.asarray(inputs["fc_b"], np.float32))

    in_maps, slot_map = [], []
    for c in range(N_CORES):
        b, r = c // 4, c % 4
        slots = [r, 7 - r, 8 + r, 15 - r]
        slot_map.append((b, slots))
        qs = q[b].reshape(NT, 128, C)[slots]
        # multiplicative causal masks per band chunk: keep iff
        # key (p + 512*i + 128*t) <= query (128*a_i + j).
        # mask[:, part, :, 0:128] = first slot's band,
        # mask[:, part, :, 128:256] = second slot's band.
        mask = np.ones((128, 2, 4, 256), np.float32)
        p = np.arange(128)[:, None, None]
        t = np.arange(4)[None, :, None]
        j = np.arange(128)[None, None, :]
        for part, (ba, cl) in enumerate(((0, 1), (2, 3))):
            mask[:, part, :, 0:128] = (
                (p + 512 * ba + 128 * t) <= (128 * slots[ba] + j))
            mask[:, part, :, 128:256] = (
                (p + 512 * cl + 128 * t) <= (128 * slots[cl] + j))
        in_maps.append({
            "q_s": np.ascontiguousarray(qs).astype(ml_dtypes.bfloat16),
            "k_f": np.ascontiguousarray(k[b].reshape(NT, 128, C)).astype(ml_dtypes.bfloat16),
            "v_f": np.ascontiguousarray(v[b].reshape(NT, 128, C)).astype(ml_dtypes.bfloat16),
            "mask": mask.astype(ml_dtypes.bfloat16),
            "vecs": vecs, "w1row": w1row,
            "cpwt": cpwt, "fcwt": fcwt, "pjwt": pjwt, "fcb": fcb,
        })
    return in_maps, slot_map


def kernel(**inputs):
    if "nc" not in _CACHE:
        _CACHE["nc"] = build()
    nc = _CACHE["nc"]
    in_maps, slot_map = _host_prep(inputs)
    res = run_bass_kernel_spmd(nc, in_maps, core_ids=list(range(N_CORES)))
    out = np.empty((B, T, C), np.float32)
    for c in range(N_CORES):
        b, slots = slot_map[c]
        o = res.results[c]["out"]
        for i, a in enumerate(slots):
            out[b, a * 128:(a + 1) * 128, :] = o[i]
    return out
